# revision 30
# baseline (speedup 1.0000x reference)
"""Trainium2 Bass kernel for nn_MultiHeadAttention (B=1, S=4096, D=2048, H=16, HD=128).

Sharding: tensor-parallel over heads — 2 heads per core on 8 NeuronCores.
Each core computes its 2 heads' Q/K/V projections, causal attention, and a
partial output projection (row-split Wo); the host sums the 8 partials and
adds the output bias (the all-reduce/unshard step).

Layout strategy (per core, all matmuls bf16 with fp32 PSUM accumulation):
  - X^T [2048, 4096] uploaded (e-major) so projections contract over e.
    DMA'd sb-major (512-seq-col blocks across all 16 e-tiles) so the first
    projection matmuls start after ~2 MB instead of after the full 16 MB.
  - Q, K produced transposed: QT/KT [d, s]. Scores computed transposed,
    S^T[k, q] = KT_tile^T @ QT, so p = exp(S^T) has k on partitions and
    attn@V needs no transpose.
  - Scores for 2 k-tiles land in one [128,1024] PSUM tile (2 banks) and get
    ONE exp instruction — halves the ACT per-instruction overhead.
  - Causal diagonal k-tiles stream only the valid q-columns (>= 128*jj into
    the q-block); the causal mask is a single shared [128,128] additive
    triangle applied via a 128-col identity matmul.
  - Softmax denominators accumulate on PE into one PSUM bank (h0 row 0,
    h1 row 32 via tile_position); 1/denom via reciprocal_approx_fast (DVE),
    broadcast across partitions with a K=1 matmul into the psF bank.
  - O-projection (row-split Wo, accumulated over both local heads) for
    q-block qb is interleaved into q-block qb+1's attention instruction
    stream, so exp-gated PE stalls are filled with ready matmuls.
  - Output partials are written bf16 (summed in fp32 on the host).

Built with bacc.Bacc (event-semaphore chains for multi-wait sync).
"""

import numpy as np
import ml_dtypes

import concourse.bass as bass
import concourse.mybir as mybir
import concourse.tile as tile
from concourse import bacc
from concourse.bass_utils import run_bass_kernel_spmd


S = 4096          # sequence length
D = 2048          # model dim
NCORES = 8
DL = D // NCORES  # 256 local head dims (2 heads)
NH = 2            # heads per core
HD = 128          # head dim
QB = 512          # q block width
NQB = S // QB     # 8
KT = 128          # k tile (partitions)
NKT = S // KT     # 32
ET = 128          # e contraction tile
NET = D // ET     # 16
NST = S // 128    # 32 s-tiles
SCALE = 1.0 / np.sqrt(HD)

BF16 = mybir.dt.bfloat16
F32 = mybir.dt.float32


def build_nc(is_causal: bool) -> bass.Bass:
    nc = bacc.Bacc()

    XT = nc.dram_tensor("xt", [D, S], BF16, kind="ExternalInput")
    WQT = nc.dram_tensor("wqt", [D, DL], BF16, kind="ExternalInput")
    WKT = nc.dram_tensor("wkt", [D, DL], BF16, kind="ExternalInput")
    WVT = nc.dram_tensor("wvt", [D, DL], BF16, kind="ExternalInput")
    # bias columns [128, 4]: bq.d0 | bq.d1 | bk.d0 | bk.d1
    BQKC = nc.dram_tensor("bqkc", [128, 4], F32, kind="ExternalInput")
    BVROW = nc.dram_tensor("bvrow", [1, DL], BF16, kind="ExternalInput")
    WOT = nc.dram_tensor("wot", [DL, D], BF16, kind="ExternalInput")
    # masks[0]: additive causal triangle (0 / -1e9); masks[1]: identity
    MASKS = nc.dram_tensor("masks", [2, 128, 128], BF16, kind="ExternalInput")
    OUT = nc.dram_tensor("out", [S, D], BF16, kind="ExternalOutput")

    with tile.TileContext(nc) as tc:
        with tc.tile_pool(name="persist", bufs=1) as persist:
            # Q head0 | Q head1 | K head0 | K head1, each [128, 4096]
            qkt = persist.tile([128, 4 * S], BF16, name="qkt")
            # V natural layout: s-tile st at cols [st*256, (st+1)*256), head h at +h*128
            vt = persist.tile([128, NST * DL], BF16, name="vt")
            ones_col = persist.tile([128, 1], BF16, name="ones_col")
            ones_row = persist.tile([1, 128], BF16, name="ones_row")
            biasqk = persist.tile([128, 4], F32, name="biasqk")
            bvrow_sb = persist.tile([1, DL], BF16, name="bvrow_sb")
            bvb_sb = persist.tile([128, DL], BF16, name="bvb_sb")
            masks_sb = persist.tile([128, 2 * 128], BF16, name="masks_sb")

            nc.vector.memset(ones_col[:, :], 1.0)
            nc.vector.memset(ones_row[:, :], 1.0)
            nc.sync.dma_start(out=bvrow_sb[:, :], in_=BVROW[:, :])
            nc.sync.dma_start(out=biasqk[:, :], in_=BQKC[:, :])
            if is_causal:
                # needed by q-block 0's diagonal matmuls — don't let it queue
                # behind the 16 MB X^T stream
                nc.sync.dma_start(
                    out=masks_sb.rearrange("p (j c) -> p j c", j=2),
                    in_=MASKS.rearrange("j p c -> p j c"),
                )

            # ---------------- Phase 2: QKV projections (sb-major) ----------
            with tc.tile_pool(name="xtp", bufs=1) as xtp, \
                 tc.tile_pool(name="wp", bufs=1) as wp, \
                 tc.tile_pool(name="ps2", bufs=3, space="PSUM") as ps2:
                xt_sb = xtp.tile([128, NET * S], BF16, name="xt_sb")
                wv_sb = wp.tile([128, NET * DL], BF16, name="wv_sb", tag="wv")
                wk_sb = wp.tile([128, NET * DL], BF16, name="wk_sb", tag="wk")
                wq_sb = wp.tile([128, NET * DL], BF16, name="wq_sb", tag="wq")
                # one strided DMA per transfer: DMA trigger instructions cost
                # ~0.6us each on the sync engine, so batching matters
                xt3 = xt_sb.rearrange("p (et s) -> p et s", et=NET)
                XT3 = XT.rearrange("(et p) s -> p et s", p=128)

                def dma_xt_block(sb):
                    nc.sync.dma_start(
                        out=xt3[:, :, sb * QB : (sb + 1) * QB],
                        in_=XT3[:, :, sb * QB : (sb + 1) * QB],
                    )

                nc.sync.dma_start(
                    out=wv_sb.rearrange("p (et d) -> p et d", et=NET),
                    in_=WVT.rearrange("(et p) d -> p et d", p=128),
                )
                # first s-block in 128-col chunks so the first V s-tile's
                # matmuls start as early as possible
                for st4 in range(4):
                    nc.sync.dma_start(
                        out=xt3[:, :, st4 * 128 : (st4 + 1) * 128],
                        in_=XT3[:, :, st4 * 128 : (st4 + 1) * 128],
                    )
                nc.sync.dma_start(
                    out=wk_sb.rearrange("p (et d) -> p et d", et=NET),
                    in_=WKT.rearrange("(et p) d -> p et d", p=128),
                )
                nc.sync.dma_start(
                    out=wq_sb.rearrange("p (et d) -> p et d", et=NET),
                    in_=WQT.rearrange("(et p) d -> p et d", p=128),
                )

                # broadcast bv across partitions once
                psb = ps2.tile([128, DL], F32, name="psb", tag="psv")
                nc.tensor.matmul(
                    psb[:, :], lhsT=ones_row[:, :], rhs=bvrow_sb[:, :],
                    start=True, stop=True,
                )
                nc.vector.tensor_copy(bvb_sb[:, :], psb[:, :])

                for sb in range(NQB):
                    if sb > 0:
                        dma_xt_block(sb)
                    # V for the 4 s-tiles of this block
                    for st4 in range(4):
                        st = 4 * sb + st4
                        psv = ps2.tile([128, DL], F32, name="psv", tag="psv")
                        for et in range(NET):
                            nc.tensor.matmul(
                                psv[:, :],
                                lhsT=xt_sb[:, et * S + st * 128 : et * S + (st + 1) * 128],
                                rhs=wv_sb[:, et * DL : (et + 1) * DL],
                                start=(et == 0),
                                stop=(et == NET - 1),
                            )
                        nc.vector.scalar_tensor_tensor(
                            out=vt[:, st * DL : (st + 1) * DL],
                            in0=psv[:, :],
                            scalar=1.0,
                            in1=bvb_sb[:, :],
                            op0=mybir.AluOpType.mult,
                            op1=mybir.AluOpType.add,
                        )
                    # K then Q for this block; bias fused into the ACT drain
                    for w_sb, base4, bias_base in (
                        (wk_sb, 2, 2), (wq_sb, 0, 0)
                    ):
                        for dt in range(NH):
                            psq = ps2.tile([128, QB], F32, name="psq", tag="psq")
                            for et in range(NET):
                                nc.tensor.matmul(
                                    psq[:, :],
                                    lhsT=w_sb[:, et * DL + dt * 128 : et * DL + (dt + 1) * 128],
                                    rhs=xt_sb[:, et * S + sb * QB : et * S + (sb + 1) * QB],
                                    start=(et == 0),
                                    stop=(et == NET - 1),
                                )
                            nc.scalar.activation(
                                qkt[:, (base4 + dt) * S + sb * QB : (base4 + dt) * S + (sb + 1) * QB],
                                psq[:, :],
                                mybir.ActivationFunctionType.Identity,
                                bias=biasqk[:, bias_base + dt : bias_base + dt + 1],
                                scale=1.0,
                            )

            # ------- Phases 3+4: attention with interleaved O-projection ---
            with tc.tile_pool(name="mid", bufs=1) as mid, \
                 tc.tile_pool(name="psO", bufs=2, space="PSUM") as psO_p, \
                 tc.tile_pool(name="psD", bufs=1, space="PSUM") as psD_p, \
                 tc.tile_pool(name="psS", bufs=2, space="PSUM") as psS_p, \
                 tc.tile_pool(name="psF", bufs=1, space="PSUM") as psF_p, \
                 tc.tile_pool(name="pp", bufs=6) as pp, \
                 tc.tile_pool(name="rp", bufs=2) as rp, \
                 tc.tile_pool(name="rbp", bufs=2) as rbp, \
                 tc.tile_pool(name="op", bufs=4) as op:
                # normalized attention outputs, transposed: (h*NQB+qb) tile [128d, 512q]
                outt = mid.tile([128, NH * NQB * QB], BF16, name="outt")
                wot_sb = mid.tile([128, NH * D], BF16, name="wot_sb")
                nc.sync.dma_start(
                    out=wot_sb.rearrange("p (h e) -> p h e", h=NH),
                    in_=WOT.rearrange("(h p) e -> p h e", p=128),
                )

                osb_open: dict = {}

                def emit_proj(qb0: int, j: int, et: int, alt: bool = False):
                    # O-projection for s-tile (qb0,j), e-chunk et; both heads
                    # accumulate in one psF bank, drained to bf16.  The four
                    # e-chunks of an s-tile share one osb staging tile so each
                    # s-tile costs a single output DMA.  In the tail (alt),
                    # items alternate into the psD bank (free after the last
                    # normalize) for a 2-deep psF rotation.
                    st = qb0 * 4 + j
                    if alt:
                        psF = psD_p.tile([128, 512], F32, name="psFt", tag="psD")
                    else:
                        psF = psF_p.tile([128, 512], F32, name="psF", tag="psF")
                    for h in range(NH):
                        o_base = (h * NQB + qb0) * QB + j * 128
                        nc.tensor.matmul(
                            psF[:, :],
                            lhsT=outt[:, o_base : o_base + 128],
                            rhs=wot_sb[:, h * D + et * 512 : h * D + (et + 1) * 512],
                            start=(h == 0),
                            stop=(h == NH - 1),
                        )
                    if st not in osb_open:
                        osb_open[st] = op.tile([128, D], BF16, name="osb", tag="osb")
                    osb = osb_open[st]
                    nc.vector.tensor_copy(
                        osb[:, et * 512 : (et + 1) * 512], psF[:, :]
                    )
                    if et == 3:
                        nc.sync.dma_start(
                            out=OUT[st * 128 : (st + 1) * 128, :],
                            in_=osb[:, :],
                        )
                        del osb_open[st]

                proj_items: list = []

                for qb in range(NQB):
                    n_k = 4 * (qb + 1) if is_causal else NKT
                    psO = {}
                    for h in range(NH):
                        psO[h] = psO_p.tile([128, QB], F32, name="psO", tag="psO")
                    psD = psD_p.tile([128, QB], F32, name="psD", tag="psD")
                    npairs = n_k // 2
                    # spread the previous q-block's O-projection through this
                    # q-block's pairs so ready matmuls fill exp-wait stalls
                    per_pair = -(-len(proj_items) // npairs) if proj_items else 0

                    for pi in range(npairs):
                        kt0 = 2 * pi
                        # scores + exp for both heads first (ACT gets a head
                        # start while the PE streams the other head's scores)
                        ps_info = {}
                        for h in range(NH):
                            diag = is_causal and (kt0 + 1 >= 4 * qb)
                            offs = (
                                (128 * (kt0 - 4 * qb), 128 * (kt0 + 1 - 4 * qb))
                                if diag
                                else (0, 0)
                            )
                            p = pp.tile([128, 2 * QB], BF16, name="p", tag="p")
                            psS = psS_p.tile([128, 2 * QB], F32, name="psS", tag="psS")
                            if not diag:
                                for u in range(2):
                                    kt = kt0 + u
                                    nc.tensor.matmul(
                                        psS[:, u * QB : (u + 1) * QB],
                                        lhsT=qkt[:, (2 + h) * S + kt * 128 : (2 + h) * S + (kt + 1) * 128],
                                        rhs=qkt[:, h * S + qb * QB : h * S + (qb + 1) * QB],
                                        start=True,
                                        stop=True,
                                    )
                                # one exp instruction for both k-tiles — ACT
                                # per-instruction overhead is ~40% of a
                                # 512-col activation
                                nc.scalar.activation(
                                    p[:, :], psS[:, :],
                                    mybir.ActivationFunctionType.Exp,
                                    scale=float(SCALE),
                                )
                            else:
                                for u in range(2):
                                    kt = kt0 + u
                                    off = offs[u]
                                    w = QB - off
                                    # scores for valid q-cols only, written at
                                    # bank-aligned offset 0 of this half
                                    nc.tensor.matmul(
                                        psS[:, u * QB : u * QB + w],
                                        lhsT=qkt[:, (2 + h) * S + kt * 128 : (2 + h) * S + (kt + 1) * 128],
                                        rhs=qkt[:, h * S + qb * QB + off : h * S + (qb + 1) * QB],
                                        start=True,
                                        stop=False,
                                    )
                                    # additive triangle on the first 128 valid
                                    # cols via identity-weight matmul
                                    nc.tensor.matmul(
                                        psS[:, u * QB : u * QB + 128],
                                        lhsT=masks_sb[:, 128:256],
                                        rhs=masks_sb[:, 0:128],
                                        start=False,
                                        stop=True,
                                    )
                                    # exp lands q-aligned in p; pad cols are
                                    # never read (attn@V / denom are narrowed)
                                    nc.scalar.activation(
                                        p[:, u * QB + off : (u + 1) * QB],
                                        psS[:, u * QB : u * QB + w],
                                        mybir.ActivationFunctionType.Exp,
                                        scale=float(SCALE),
                                    )
                            ps_info[h] = (p, offs)
                        # a slice of the deferred O-projection fills the gap
                        # between score issue and exp completion
                        for _ in range(per_pair):
                            if proj_items:
                                emit_proj(*proj_items.pop(0))
                        # attn@V + denominators, narrowed to the causally
                        # valid q-columns on diagonal k-tiles
                        for h in range(NH):
                            p, offs = ps_info[h]
                            for u in range(2):
                                kt = kt0 + u
                                off = offs[u]
                                nc.tensor.matmul(
                                    psO[h][:, off:QB],
                                    lhsT=vt[:, kt * DL + h * 128 : kt * DL + (h + 1) * 128],
                                    rhs=p[:, u * QB + off : (u + 1) * QB],
                                    start=(kt == 0),
                                    stop=(kt == n_k - 1),
                                )
                            # denominator: DVE pre-sums the two k-tiles (bf16,
                            # off the critical path), then ONE ones-matmul per
                            # pair — halves the PE denominator column count
                            off0, off1 = offs
                            psum2 = pp.tile([128, QB], BF16, name="psum2", tag="ps2", bufs=3)
                            if off1 > off0:
                                # u0's exclusive strip, then the common range
                                nc.vector.tensor_copy(
                                    psum2[:, off0:off1], p[:, off0:off1]
                                )
                                nc.vector.tensor_add(
                                    psum2[:, off1:QB],
                                    p[:, off1:QB],
                                    p[:, QB + off1 : 2 * QB],
                                )
                            else:
                                nc.vector.tensor_add(
                                    psum2[:, :], p[:, 0:QB], p[:, QB : 2 * QB]
                                )
                            nc.tensor.matmul(
                                psD[32 * h : 32 * h + 1, off0:QB],
                                lhsT=ones_col[:, :],
                                rhs=psum2[:, off0:QB],
                                start=(kt0 == 0),
                                stop=(kt0 + 2 >= n_k),
                                tile_position=(0, 32 * h),
                            )

                    # normalize: 1/denom (fast approx), broadcast via K=1
                    # matmul into the psF bank, scale psO into outt (bf16)
                    for h in range(NH):
                        # normalize chain stays off the ACT engine — ACT is
                        # the attention-phase pacer (exp).  NOTE: the custom
                        # reciprocal_approx_fast DVE op reads garbage from
                        # PSUM on hardware (CoreSim accepts it) — stage the
                        # denominator row through SBUF first.
                        dsb = rp.tile([1, QB], F32, name="dsb", tag="dsb")
                        nc.vector.tensor_copy(dsb[:, :], psD[32 * h : 32 * h + 1, :])
                        recipf = rp.tile([1, QB], F32, name="recipf", tag="recipf")
                        nc.vector.reciprocal_approx_fast(
                            out=recipf[:, :], in_=dsb[:, :]
                        )
                        recipb = rp.tile([1, QB], BF16, name="recipb", tag="recipb")
                        nc.vector.tensor_copy(recipb[:, :], recipf[:, :])
                        psB = psF_p.tile([128, QB], F32, name="psB", tag="psF")
                        nc.tensor.matmul(
                            psB[:, :], lhsT=ones_row[:, :], rhs=recipb[:, :],
                            start=True, stop=True,
                        )
                        rb = rbp.tile([128, QB], F32, name="rb", tag="rb")
                        # ACT has a lull at the q-block boundary; copying here
                        # overlaps the DVE half of the other head's chain
                        nc.scalar.copy(rb[:, :], psB[:, :])
                        o_base = (h * NQB + qb) * QB
                        nc.vector.tensor_mul(
                            outt[:, o_base : o_base + QB], psO[h][:, :], rb[:, :]
                        )
                    # flush any leftovers, then queue this block's O-proj
                    while proj_items:
                        emit_proj(*proj_items.pop(0))
                    proj_items = [(qb, j, et) for j in range(4) for et in range(4)]

                # tail: O-projection of the last q-block, alternating PSUM
                # banks so drains overlap the next pair of matmuls
                ti = 0
                while proj_items:
                    emit_proj(*proj_items.pop(0), alt=(ti % 2 == 1))
                    ti += 1
    nc.finalize()
    return nc


def _bf16(a: np.ndarray) -> np.ndarray:
    return np.ascontiguousarray(a.astype(ml_dtypes.bfloat16))


def make_in_maps(X, Wq, bq, Wk, bk, Wv, bv, Wo, is_causal: bool):
    x2d = np.asarray(X, dtype=np.float32).reshape(S, D)
    xt = _bf16(x2d.T)
    masks = np.zeros((2, 128, 128), dtype=ml_dtypes.bfloat16)
    if is_causal:
        ki = np.arange(128)[:, None]
        cj = np.arange(128)[None, :]
        masks[0] = np.where(ki <= cj, 0.0, -1e9).astype(ml_dtypes.bfloat16)
        masks[1] = np.eye(128, dtype=ml_dtypes.bfloat16)

    in_maps = []
    for c in range(NCORES):
        sl = slice(c * DL, (c + 1) * DL)
        in_maps.append(
            {
                "xt": xt,
                "wqt": _bf16(np.asarray(Wq)[sl, :].T),
                "wkt": _bf16(np.asarray(Wk)[sl, :].T),
                "wvt": _bf16(np.asarray(Wv)[sl, :].T),
                "bqkc": np.ascontiguousarray(
                    np.stack(
                        [
                            np.asarray(bq, dtype=np.float32)[sl][:128],
                            np.asarray(bq, dtype=np.float32)[sl][128:],
                            np.asarray(bk, dtype=np.float32)[sl][:128],
                            np.asarray(bk, dtype=np.float32)[sl][128:],
                        ],
                        axis=1,
                    )
                ),
                "bvrow": _bf16(np.asarray(bv)[None, sl]),
                "wot": _bf16(np.asarray(Wo)[:, sl].T),
                "masks": masks,
            }
        )
    return in_maps


_NC_CACHE: dict = {}


def _get_nc(is_causal: bool) -> bass.Bass:
    if is_causal not in _NC_CACHE:
        _NC_CACHE[is_causal] = build_nc(is_causal)
    return _NC_CACHE[is_causal]


def kernel(X, Wq, bq, Wk, bk, Wv, bv, Wo, bo, is_causal, **run_kwargs):
    causal = bool(int(np.asarray(is_causal)))
    nc = _get_nc(causal)
    in_maps = make_in_maps(X, Wq, bq, Wk, bk, Wv, bv, Wo, causal)
    res = run_bass_kernel_spmd(nc, in_maps, core_ids=list(range(NCORES)), **run_kwargs)
    out = np.asarray(bo, dtype=np.float32)[None, :].repeat(S, axis=0)
    for c in range(NCORES):
        out += np.asarray(res.results[c]["out"], dtype=np.float32)
    return out.reshape(1, S, D)


# revision 31
# speedup vs baseline: 1.1747x; 1.1747x over previous
"""Trainium2 Bass kernel for nn_MultiHeadAttention (B=1, S=4096, D=2048, H=16, HD=128).

Sharding: tensor-parallel over heads — 2 heads per core on 8 NeuronCores.
Each core computes its 2 heads' Q/K/V projections, causal attention, and a
partial output projection (row-split Wo); the host sums the 8 partials and
adds the output bias (the all-reduce/unshard step).

Layout strategy (per core, all matmuls bf16 with fp32 PSUM accumulation):
  - X^T [2048, 4096] uploaded (e-major) so projections contract over e.
    DMA'd sb-major (512-seq-col blocks across all 16 e-tiles) so the first
    projection matmuls start after ~2 MB instead of after the full 16 MB.
  - Q, K produced transposed: QT/KT [d, s]. Scores computed transposed,
    S^T[k, q] = KT_tile^T @ QT, so p = exp(S^T) has k on partitions and
    attn@V needs no transpose.
  - Scores for 2 k-tiles land in one [128,1024] PSUM tile (2 banks) and get
    ONE exp instruction — halves the ACT per-instruction overhead.
  - Causal diagonal k-tiles stream only the valid q-columns (>= 128*jj into
    the q-block); the causal mask is a single shared [128,128] additive
    triangle applied via a 128-col identity matmul.
  - Softmax denominators accumulate on PE into one PSUM bank (h0 row 0,
    h1 row 32 via tile_position); 1/denom via reciprocal_approx_fast (DVE),
    broadcast across partitions with a K=1 matmul into the psF bank.
  - O-projection (row-split Wo, accumulated over both local heads) for
    q-block qb is interleaved into q-block qb+1's attention instruction
    stream, so exp-gated PE stalls are filled with ready matmuls.
  - Output partials are written bf16 (summed in fp32 on the host).

Built with bacc.Bacc (event-semaphore chains for multi-wait sync).
"""

import numpy as np
import ml_dtypes

import concourse.bass as bass
import concourse.mybir as mybir
import concourse.tile as tile
from concourse import bacc
from concourse.bass_utils import run_bass_kernel_spmd


S = 4096          # sequence length
D = 2048          # model dim
NCORES = 8
DL = D // NCORES  # 256 local head dims (2 heads)
NH = 2            # heads per core
HD = 128          # head dim
QB = 512          # q block width
NQB = S // QB     # 8
KT = 128          # k tile (partitions)
NKT = S // KT     # 32
ET = 128          # e contraction tile
NET = D // ET     # 16
NST = S // 128    # 32 s-tiles
SCALE = 1.0 / np.sqrt(HD)

BF16 = mybir.dt.bfloat16
F32 = mybir.dt.float32


def build_nc(is_causal: bool) -> bass.Bass:
    nc = bacc.Bacc()

    XT = nc.dram_tensor("xt", [D, S], BF16, kind="ExternalInput")
    WQT = nc.dram_tensor("wqt", [D, DL], BF16, kind="ExternalInput")
    WKT = nc.dram_tensor("wkt", [D, DL], BF16, kind="ExternalInput")
    WVT = nc.dram_tensor("wvt", [D, DL], BF16, kind="ExternalInput")
    # bias columns [128, 4]: bq.d0 | bq.d1 | bk.d0 | bk.d1
    BQKC = nc.dram_tensor("bqkc", [128, 4], F32, kind="ExternalInput")
    BVROW = nc.dram_tensor("bvrow", [1, DL], BF16, kind="ExternalInput")
    WOT = nc.dram_tensor("wot", [DL, D], BF16, kind="ExternalInput")
    # masks[0]: additive causal triangle (0 / -1e9); masks[1]: identity
    MASKS = nc.dram_tensor("masks", [2, 128, 128], BF16, kind="ExternalInput")
    OUT = nc.dram_tensor("out", [S, D], BF16, kind="ExternalOutput")

    with tile.TileContext(nc) as tc:
        with tc.tile_pool(name="persist", bufs=1) as persist:
            # Q head0 | Q head1 | K head0 | K head1, each [128, 4096]
            qkt = persist.tile([128, 4 * S], BF16, name="qkt")
            # V natural layout: s-tile st at cols [st*256, (st+1)*256), head h at +h*128
            vt = persist.tile([128, NST * DL], BF16, name="vt")
            ones_col = persist.tile([128, 1], BF16, name="ones_col")
            ones_row = persist.tile([1, 128], BF16, name="ones_row")
            biasqk = persist.tile([128, 4], F32, name="biasqk")
            bvrow_sb = persist.tile([1, DL], BF16, name="bvrow_sb")
            bvb_sb = persist.tile([128, DL], BF16, name="bvb_sb")
            masks_sb = persist.tile([128, 2 * 128], BF16, name="masks_sb")

            nc.vector.memset(ones_col[:, :], 1.0)
            nc.vector.memset(ones_row[:, :], 1.0)
            nc.sync.dma_start(out=bvrow_sb[:, :], in_=BVROW[:, :])
            nc.sync.dma_start(out=biasqk[:, :], in_=BQKC[:, :])
            if is_causal:
                # needed by q-block 0's diagonal matmuls — don't let it queue
                # behind the 16 MB X^T stream
                nc.sync.dma_start(
                    out=masks_sb.rearrange("p (j c) -> p j c", j=2),
                    in_=MASKS.rearrange("j p c -> p j c"),
                )

            # ---------------- Phase 2: QKV projections (sb-major) ----------
            with tc.tile_pool(name="xtp", bufs=1) as xtp, \
                 tc.tile_pool(name="wp", bufs=1) as wp, \
                 tc.tile_pool(name="ps2", bufs=3, space="PSUM") as ps2:
                xt_sb = xtp.tile([128, NET * S], BF16, name="xt_sb")
                wv_sb = wp.tile([128, NET * DL], BF16, name="wv_sb", tag="wv")
                wk_sb = wp.tile([128, NET * DL], BF16, name="wk_sb", tag="wk")
                wq_sb = wp.tile([128, NET * DL], BF16, name="wq_sb", tag="wq")
                # one strided DMA per transfer: DMA trigger instructions cost
                # ~0.6us each on the sync engine, so batching matters
                xt3 = xt_sb.rearrange("p (et s) -> p et s", et=NET)
                XT3 = XT.rearrange("(et p) s -> p et s", p=128)

                def dma_xt_block(sb):
                    nc.sync.dma_start(
                        out=xt3[:, :, sb * QB : (sb + 1) * QB],
                        in_=XT3[:, :, sb * QB : (sb + 1) * QB],
                    )

                nc.sync.dma_start(
                    out=wv_sb.rearrange("p (et d) -> p et d", et=NET),
                    in_=WVT.rearrange("(et p) d -> p et d", p=128),
                )
                # first s-block in 128-col chunks so the first V s-tile's
                # matmuls start as early as possible
                for st4 in range(4):
                    nc.sync.dma_start(
                        out=xt3[:, :, st4 * 128 : (st4 + 1) * 128],
                        in_=XT3[:, :, st4 * 128 : (st4 + 1) * 128],
                    )
                nc.sync.dma_start(
                    out=wk_sb.rearrange("p (et d) -> p et d", et=NET),
                    in_=WKT.rearrange("(et p) d -> p et d", p=128),
                )
                nc.sync.dma_start(
                    out=wq_sb.rearrange("p (et d) -> p et d", et=NET),
                    in_=WQT.rearrange("(et p) d -> p et d", p=128),
                )

                # broadcast bv across partitions once
                psb = ps2.tile([128, DL], F32, name="psb", tag="psv")
                nc.tensor.matmul(
                    psb[:, :], lhsT=ones_row[:, :], rhs=bvrow_sb[:, :],
                    start=True, stop=True,
                )
                nc.vector.tensor_copy(bvb_sb[:, :], psb[:, :])

                for sb in range(NQB):
                    if sb > 0:
                        dma_xt_block(sb)
                    # V for the 4 s-tiles of this block
                    for st4 in range(4):
                        st = 4 * sb + st4
                        psv = ps2.tile([128, DL], F32, name="psv", tag="psv")
                        for et in range(NET):
                            nc.tensor.matmul(
                                psv[:, :],
                                lhsT=xt_sb[:, et * S + st * 128 : et * S + (st + 1) * 128],
                                rhs=wv_sb[:, et * DL : (et + 1) * DL],
                                start=(et == 0),
                                stop=(et == NET - 1),
                            )
                        nc.vector.scalar_tensor_tensor(
                            out=vt[:, st * DL : (st + 1) * DL],
                            in0=psv[:, :],
                            scalar=1.0,
                            in1=bvb_sb[:, :],
                            op0=mybir.AluOpType.mult,
                            op1=mybir.AluOpType.add,
                        )
                    # K then Q for this block; bias fused into the ACT drain
                    for w_sb, base4, bias_base in (
                        (wk_sb, 2, 2), (wq_sb, 0, 0)
                    ):
                        for dt in range(NH):
                            psq = ps2.tile([128, QB], F32, name="psq", tag="psq")
                            for et in range(NET):
                                nc.tensor.matmul(
                                    psq[:, :],
                                    lhsT=w_sb[:, et * DL + dt * 128 : et * DL + (dt + 1) * 128],
                                    rhs=xt_sb[:, et * S + sb * QB : et * S + (sb + 1) * QB],
                                    start=(et == 0),
                                    stop=(et == NET - 1),
                                )
                            nc.scalar.activation(
                                qkt[:, (base4 + dt) * S + sb * QB : (base4 + dt) * S + (sb + 1) * QB],
                                psq[:, :],
                                mybir.ActivationFunctionType.Identity,
                                bias=biasqk[:, bias_base + dt : bias_base + dt + 1],
                                scale=1.0,
                            )

            # ------- Phases 3+4: attention with interleaved O-projection ---
            with tc.tile_pool(name="mid", bufs=1) as mid, \
                 tc.tile_pool(name="psO", bufs=2, space="PSUM") as psO_p, \
                 tc.tile_pool(name="psD", bufs=1, space="PSUM") as psD_p, \
                 tc.tile_pool(name="psS", bufs=2, space="PSUM") as psS_p, \
                 tc.tile_pool(name="psF", bufs=1, space="PSUM") as psF_p, \
                 tc.tile_pool(name="pp", bufs=6) as pp, \
                 tc.tile_pool(name="rp", bufs=2) as rp, \
                 tc.tile_pool(name="rbp", bufs=2) as rbp, \
                 tc.tile_pool(name="op", bufs=4) as op:
                # normalized attention outputs, transposed: (h*NQB+qb) tile [128d, 512q]
                outt = mid.tile([128, NH * NQB * QB], BF16, name="outt")
                wot_sb = mid.tile([128, NH * D], BF16, name="wot_sb")
                nc.sync.dma_start(
                    out=wot_sb.rearrange("p (h e) -> p h e", h=NH),
                    in_=WOT.rearrange("(h p) e -> p h e", p=128),
                )

                osb_open: dict = {}

                def emit_proj(qb0: int, j: int, et: int, alt: bool = False):
                    # O-projection for s-tile (qb0,j), e-chunk et; both heads
                    # accumulate in one psF bank, drained to bf16.  The four
                    # e-chunks of an s-tile share one osb staging tile so each
                    # s-tile costs a single output DMA.  In the tail (alt),
                    # items alternate into the psD bank (free after the last
                    # normalize) for a 2-deep psF rotation.
                    st = qb0 * 4 + j
                    if alt:
                        psF = psD_p.tile([128, 512], F32, name="psFt", tag="psD")
                    else:
                        psF = psF_p.tile([128, 512], F32, name="psF", tag="psF")
                    for h in range(NH):
                        o_base = (h * NQB + qb0) * QB + j * 128
                        nc.tensor.matmul(
                            psF[:, :],
                            lhsT=outt[:, o_base : o_base + 128],
                            rhs=wot_sb[:, h * D + et * 512 : h * D + (et + 1) * 512],
                            start=(h == 0),
                            stop=(h == NH - 1),
                        )
                    if st not in osb_open:
                        osb_open[st] = op.tile([128, D], BF16, name="osb", tag="osb")
                    osb = osb_open[st]
                    nc.vector.tensor_copy(
                        osb[:, et * 512 : (et + 1) * 512], psF[:, :]
                    )
                    if et == 3:
                        nc.sync.dma_start(
                            out=OUT[st * 128 : (st + 1) * 128, :],
                            in_=osb[:, :],
                        )
                        del osb_open[st]

                proj_items: list = []

                for qb in range(NQB):
                    n_k = 4 * (qb + 1) if is_causal else NKT
                    psO = {}
                    for h in range(NH):
                        psO[h] = psO_p.tile([128, QB], F32, name="psO", tag="psO")
                    psD = psD_p.tile([128, QB], F32, name="psD", tag="psD")
                    npairs = n_k // 2
                    # spread the previous q-block's O-projection through this
                    # q-block's pairs so ready matmuls fill exp-wait stalls
                    per_pair = -(-len(proj_items) // npairs) if proj_items else 0

                    for pi in range(npairs):
                        kt0 = 2 * pi
                        # scores + exp for both heads first (ACT gets a head
                        # start while the PE streams the other head's scores)
                        ps_info = {}
                        for h in range(NH):
                            diag = is_causal and (kt0 + 1 >= 4 * qb)
                            offs = (
                                (128 * (kt0 - 4 * qb), 128 * (kt0 + 1 - 4 * qb))
                                if diag
                                else (0, 0)
                            )
                            p = pp.tile([128, 2 * QB], BF16, name="p", tag="p")
                            psS = psS_p.tile([128, 2 * QB], F32, name="psS", tag="psS")
                            if not diag:
                                for u in range(2):
                                    kt = kt0 + u
                                    nc.tensor.matmul(
                                        psS[:, u * QB : (u + 1) * QB],
                                        lhsT=qkt[:, (2 + h) * S + kt * 128 : (2 + h) * S + (kt + 1) * 128],
                                        rhs=qkt[:, h * S + qb * QB : h * S + (qb + 1) * QB],
                                        start=True,
                                        stop=True,
                                    )
                                # one exp instruction for both k-tiles — ACT
                                # per-instruction overhead is ~40% of a
                                # 512-col activation
                                nc.scalar.activation(
                                    p[:, :], psS[:, :],
                                    mybir.ActivationFunctionType.Exp,
                                    scale=float(SCALE),
                                )
                            else:
                                for u in range(2):
                                    kt = kt0 + u
                                    off = offs[u]
                                    w = QB - off
                                    # scores for valid q-cols only, written at
                                    # bank-aligned offset 0 of this half
                                    nc.tensor.matmul(
                                        psS[:, u * QB : u * QB + w],
                                        lhsT=qkt[:, (2 + h) * S + kt * 128 : (2 + h) * S + (kt + 1) * 128],
                                        rhs=qkt[:, h * S + qb * QB + off : h * S + (qb + 1) * QB],
                                        start=True,
                                        stop=False,
                                    )
                                    # additive triangle on the first 128 valid
                                    # cols via identity-weight matmul
                                    nc.tensor.matmul(
                                        psS[:, u * QB : u * QB + 128],
                                        lhsT=masks_sb[:, 128:256],
                                        rhs=masks_sb[:, 0:128],
                                        start=False,
                                        stop=True,
                                    )
                                    # exp lands q-aligned in p; pad cols are
                                    # never read (attn@V / denom are narrowed)
                                    nc.scalar.activation(
                                        p[:, u * QB + off : (u + 1) * QB],
                                        psS[:, u * QB : u * QB + w],
                                        mybir.ActivationFunctionType.Exp,
                                        scale=float(SCALE),
                                    )
                            ps_info[h] = (p, offs)
                        # a slice of the deferred O-projection fills the gap
                        # between score issue and exp completion
                        for _ in range(per_pair):
                            if proj_items:
                                emit_proj(*proj_items.pop(0))
                        # attn@V + denominators, narrowed to the causally
                        # valid q-columns on diagonal k-tiles
                        for h in range(NH):
                            p, offs = ps_info[h]
                            for u in range(2):
                                kt = kt0 + u
                                off = offs[u]
                                nc.tensor.matmul(
                                    psO[h][:, off:QB],
                                    lhsT=vt[:, kt * DL + h * 128 : kt * DL + (h + 1) * 128],
                                    rhs=p[:, u * QB + off : (u + 1) * QB],
                                    start=(kt == 0),
                                    stop=(kt == n_k - 1),
                                )
                            # denominator: DVE pre-sums the two k-tiles (bf16,
                            # off the critical path), then ONE ones-matmul per
                            # pair — halves the PE denominator column count
                            off0, off1 = offs
                            psum2 = pp.tile([128, QB], BF16, name="psum2", tag="ps2", bufs=3)
                            if off1 > off0:
                                # u0's exclusive strip, then the common range
                                nc.vector.tensor_copy(
                                    psum2[:, off0:off1], p[:, off0:off1]
                                )
                                nc.vector.tensor_add(
                                    psum2[:, off1:QB],
                                    p[:, off1:QB],
                                    p[:, QB + off1 : 2 * QB],
                                )
                            else:
                                nc.vector.tensor_add(
                                    psum2[:, :], p[:, 0:QB], p[:, QB : 2 * QB]
                                )
                            nc.tensor.matmul(
                                psD[32 * h : 32 * h + 1, off0:QB],
                                lhsT=ones_col[:, :],
                                rhs=psum2[:, off0:QB],
                                start=(kt0 == 0),
                                stop=(kt0 + 2 >= n_k),
                                tile_position=(0, 32 * h),
                            )

                    # normalize: 1/denom (fast approx), broadcast via K=1
                    # matmul into the psF bank, scale psO into outt (bf16)
                    for h in range(NH):
                        # normalize chain stays off the ACT engine — ACT is
                        # the attention-phase pacer (exp).  NOTE: the custom
                        # reciprocal_approx_fast DVE op reads garbage from
                        # PSUM on hardware (CoreSim accepts it) — stage the
                        # denominator row through SBUF first.
                        dsb = rp.tile([1, QB], F32, name="dsb", tag="dsb")
                        nc.vector.tensor_copy(dsb[:, :], psD[32 * h : 32 * h + 1, :])
                        recipf = rp.tile([1, QB], F32, name="recipf", tag="recipf")
                        nc.vector.reciprocal_approx_fast(
                            out=recipf[:, :], in_=dsb[:, :]
                        )
                        recipb = rp.tile([1, QB], BF16, name="recipb", tag="recipb")
                        nc.vector.tensor_copy(recipb[:, :], recipf[:, :])
                        psB = psF_p.tile([128, QB], F32, name="psB", tag="psF")
                        nc.tensor.matmul(
                            psB[:, :], lhsT=ones_row[:, :], rhs=recipb[:, :],
                            start=True, stop=True,
                        )
                        rb = rbp.tile([128, QB], F32, name="rb", tag="rb")
                        # keep this on DVE: an ACT copy here head-of-line
                        # blocks the in-order ACT queue (and all later exps)
                        # behind the whole normalize chain
                        nc.vector.tensor_copy(rb[:, :], psB[:, :])
                        o_base = (h * NQB + qb) * QB
                        nc.vector.tensor_mul(
                            outt[:, o_base : o_base + QB], psO[h][:, :], rb[:, :]
                        )
                    # flush any leftovers, then queue this block's O-proj
                    while proj_items:
                        emit_proj(*proj_items.pop(0))
                    proj_items = [(qb, j, et) for j in range(4) for et in range(4)]

                # tail: O-projection of the last q-block, alternating PSUM
                # banks so drains overlap the next pair of matmuls
                ti = 0
                while proj_items:
                    emit_proj(*proj_items.pop(0), alt=(ti % 2 == 1))
                    ti += 1
    nc.finalize()
    return nc


def _bf16(a: np.ndarray) -> np.ndarray:
    return np.ascontiguousarray(a.astype(ml_dtypes.bfloat16))


def make_in_maps(X, Wq, bq, Wk, bk, Wv, bv, Wo, is_causal: bool):
    x2d = np.asarray(X, dtype=np.float32).reshape(S, D)
    xt = _bf16(x2d.T)
    masks = np.zeros((2, 128, 128), dtype=ml_dtypes.bfloat16)
    if is_causal:
        ki = np.arange(128)[:, None]
        cj = np.arange(128)[None, :]
        masks[0] = np.where(ki <= cj, 0.0, -1e9).astype(ml_dtypes.bfloat16)
        masks[1] = np.eye(128, dtype=ml_dtypes.bfloat16)

    in_maps = []
    for c in range(NCORES):
        sl = slice(c * DL, (c + 1) * DL)
        in_maps.append(
            {
                "xt": xt,
                "wqt": _bf16(np.asarray(Wq)[sl, :].T),
                "wkt": _bf16(np.asarray(Wk)[sl, :].T),
                "wvt": _bf16(np.asarray(Wv)[sl, :].T),
                "bqkc": np.ascontiguousarray(
                    np.stack(
                        [
                            np.asarray(bq, dtype=np.float32)[sl][:128],
                            np.asarray(bq, dtype=np.float32)[sl][128:],
                            np.asarray(bk, dtype=np.float32)[sl][:128],
                            np.asarray(bk, dtype=np.float32)[sl][128:],
                        ],
                        axis=1,
                    )
                ),
                "bvrow": _bf16(np.asarray(bv)[None, sl]),
                "wot": _bf16(np.asarray(Wo)[:, sl].T),
                "masks": masks,
            }
        )
    return in_maps


_NC_CACHE: dict = {}


def _get_nc(is_causal: bool) -> bass.Bass:
    if is_causal not in _NC_CACHE:
        _NC_CACHE[is_causal] = build_nc(is_causal)
    return _NC_CACHE[is_causal]


def kernel(X, Wq, bq, Wk, bk, Wv, bv, Wo, bo, is_causal, **run_kwargs):
    causal = bool(int(np.asarray(is_causal)))
    nc = _get_nc(causal)
    in_maps = make_in_maps(X, Wq, bq, Wk, bk, Wv, bv, Wo, causal)
    res = run_bass_kernel_spmd(nc, in_maps, core_ids=list(range(NCORES)), **run_kwargs)
    out = np.asarray(bo, dtype=np.float32)[None, :].repeat(S, axis=0)
    for c in range(NCORES):
        out += np.asarray(res.results[c]["out"], dtype=np.float32)
    return out.reshape(1, S, D)


# revision 34
# speedup vs baseline: 1.2007x; 1.0221x over previous
"""Trainium2 Bass kernel for nn_MultiHeadAttention (B=1, S=4096, D=2048, H=16, HD=128).

Sharding: tensor-parallel over heads — 2 heads per core on 8 NeuronCores.
Each core computes its 2 heads' Q/K/V projections, causal attention, and a
partial output projection (row-split Wo); the host sums the 8 partials and
adds the output bias (the all-reduce/unshard step).

Layout strategy (per core, all matmuls bf16 with fp32 PSUM accumulation):
  - X^T [2048, 4096] uploaded (e-major) so projections contract over e.
    DMA'd sb-major (512-seq-col blocks across all 16 e-tiles) so the first
    projection matmuls start after ~2 MB instead of after the full 16 MB.
  - Q, K produced transposed: QT/KT [d, s]. Scores computed transposed,
    S^T[k, q] = KT_tile^T @ QT, so p = exp(S^T) has k on partitions and
    attn@V needs no transpose.
  - Scores for 2 k-tiles land in one [128,1024] PSUM tile (2 banks) and get
    ONE exp instruction — halves the ACT per-instruction overhead.
  - Causal diagonal k-tiles stream only the valid q-columns (>= 128*jj into
    the q-block); the causal mask is a single shared [128,128] additive
    triangle applied via a 128-col identity matmul.
  - Softmax denominators accumulate on PE into one PSUM bank (h0 row 0,
    h1 row 32 via tile_position); 1/denom via reciprocal_approx_fast (DVE),
    broadcast across partitions with a K=1 matmul into the psF bank.
  - O-projection (row-split Wo, accumulated over both local heads) for
    q-block qb is interleaved into q-block qb+1's attention instruction
    stream, so exp-gated PE stalls are filled with ready matmuls.
  - Output partials are written bf16 (summed in fp32 on the host).

Built with bacc.Bacc (event-semaphore chains for multi-wait sync).
"""

import numpy as np
import ml_dtypes

import concourse.bass as bass
import concourse.mybir as mybir
import concourse.tile as tile
from concourse import bacc
from concourse.bass_utils import run_bass_kernel_spmd


S = 4096          # sequence length
D = 2048          # model dim
NCORES = 8
DL = D // NCORES  # 256 local head dims (2 heads)
NH = 2            # heads per core
HD = 128          # head dim
QB = 512          # q block width
NQB = S // QB     # 8
KT = 128          # k tile (partitions)
NKT = S // KT     # 32
ET = 128          # e contraction tile
NET = D // ET     # 16
NST = S // 128    # 32 s-tiles
SCALE = 1.0 / np.sqrt(HD)

BF16 = mybir.dt.bfloat16
F32 = mybir.dt.float32


def build_nc(is_causal: bool) -> bass.Bass:
    nc = bacc.Bacc()

    XT = nc.dram_tensor("xt", [D, S], BF16, kind="ExternalInput")
    WQT = nc.dram_tensor("wqt", [D, DL], BF16, kind="ExternalInput")
    WKT = nc.dram_tensor("wkt", [D, DL], BF16, kind="ExternalInput")
    WVT = nc.dram_tensor("wvt", [D, DL], BF16, kind="ExternalInput")
    # bias columns [128, 4]: bq.d0 | bq.d1 | bk.d0 | bk.d1
    BQKC = nc.dram_tensor("bqkc", [128, 4], F32, kind="ExternalInput")
    BVROW = nc.dram_tensor("bvrow", [1, DL], BF16, kind="ExternalInput")
    WOT = nc.dram_tensor("wot", [DL, D], BF16, kind="ExternalInput")
    # masks[0]: additive causal triangle (0 / -1e9); masks[1]: identity
    MASKS = nc.dram_tensor("masks", [2, 128, 128], BF16, kind="ExternalInput")
    OUT = nc.dram_tensor("out", [S, D], BF16, kind="ExternalOutput")

    with tile.TileContext(nc) as tc:
        with tc.tile_pool(name="persist", bufs=1) as persist:
            # Q head0 | Q head1 | K head0 | K head1, each [128, 4096]
            qkt = persist.tile([128, 4 * S], BF16, name="qkt")
            # V natural layout: s-tile st at cols [st*256, (st+1)*256), head h at +h*128
            vt = persist.tile([128, NST * DL], BF16, name="vt")
            ones_col = persist.tile([128, 1], BF16, name="ones_col")
            ones_row = persist.tile([1, 128], BF16, name="ones_row")
            biasqk = persist.tile([128, 4], F32, name="biasqk")
            bvrow_sb = persist.tile([1, DL], BF16, name="bvrow_sb")
            bvb_sb = persist.tile([128, DL], BF16, name="bvb_sb")
            masks_sb = persist.tile([128, 2 * 128], BF16, name="masks_sb")

            nc.vector.memset(ones_col[:, :], 1.0)
            nc.vector.memset(ones_row[:, :], 1.0)
            nc.sync.dma_start(out=bvrow_sb[:, :], in_=BVROW[:, :])
            nc.sync.dma_start(out=biasqk[:, :], in_=BQKC[:, :])
            if is_causal:
                # needed by q-block 0's diagonal matmuls — don't let it queue
                # behind the 16 MB X^T stream
                nc.sync.dma_start(
                    out=masks_sb.rearrange("p (j c) -> p j c", j=2),
                    in_=MASKS.rearrange("j p c -> p j c"),
                )

            # ---------------- Phase 2: QKV projections (sb-major) ----------
            with tc.tile_pool(name="xtp", bufs=1) as xtp, \
                 tc.tile_pool(name="wp", bufs=1) as wp, \
                 tc.tile_pool(name="ps2", bufs=3, space="PSUM") as ps2:
                xt_sb = xtp.tile([128, NET * S], BF16, name="xt_sb")
                wv_sb = wp.tile([128, NET * DL], BF16, name="wv_sb", tag="wv")
                wk_sb = wp.tile([128, NET * DL], BF16, name="wk_sb", tag="wk")
                wq_sb = wp.tile([128, NET * DL], BF16, name="wq_sb", tag="wq")
                # one strided DMA per transfer: DMA trigger instructions cost
                # ~0.6us each on the sync engine, so batching matters
                xt3 = xt_sb.rearrange("p (et s) -> p et s", et=NET)
                XT3 = XT.rearrange("(et p) s -> p et s", p=128)

                def dma_xt_block(sb):
                    nc.sync.dma_start(
                        out=xt3[:, :, sb * QB : (sb + 1) * QB],
                        in_=XT3[:, :, sb * QB : (sb + 1) * QB],
                    )

                nc.sync.dma_start(
                    out=wv_sb.rearrange("p (et d) -> p et d", et=NET),
                    in_=WVT.rearrange("(et p) d -> p et d", p=128),
                )
                # first s-block in 128-col chunks so the first V s-tile's
                # matmuls start as early as possible
                for st4 in range(4):
                    nc.sync.dma_start(
                        out=xt3[:, :, st4 * 128 : (st4 + 1) * 128],
                        in_=XT3[:, :, st4 * 128 : (st4 + 1) * 128],
                    )
                nc.sync.dma_start(
                    out=wk_sb.rearrange("p (et d) -> p et d", et=NET),
                    in_=WKT.rearrange("(et p) d -> p et d", p=128),
                )
                nc.sync.dma_start(
                    out=wq_sb.rearrange("p (et d) -> p et d", et=NET),
                    in_=WQT.rearrange("(et p) d -> p et d", p=128),
                )

                # broadcast bv across partitions once
                psb = ps2.tile([128, DL], F32, name="psb", tag="psv")
                nc.tensor.matmul(
                    psb[:, :], lhsT=ones_row[:, :], rhs=bvrow_sb[:, :],
                    start=True, stop=True,
                )
                nc.vector.tensor_copy(bvb_sb[:, :], psb[:, :])

                for sb in range(NQB):
                    if sb > 0:
                        dma_xt_block(sb)
                    # V for the 4 s-tiles of this block
                    for st4 in range(4):
                        st = 4 * sb + st4
                        psv = ps2.tile([128, DL], F32, name="psv", tag="psv")
                        for et in range(NET):
                            nc.tensor.matmul(
                                psv[:, :],
                                lhsT=xt_sb[:, et * S + st * 128 : et * S + (st + 1) * 128],
                                rhs=wv_sb[:, et * DL : (et + 1) * DL],
                                start=(et == 0),
                                stop=(et == NET - 1),
                            )
                        nc.vector.scalar_tensor_tensor(
                            out=vt[:, st * DL : (st + 1) * DL],
                            in0=psv[:, :],
                            scalar=1.0,
                            in1=bvb_sb[:, :],
                            op0=mybir.AluOpType.mult,
                            op1=mybir.AluOpType.add,
                        )
                    # K then Q for this block; bias fused into the ACT drain
                    for w_sb, base4, bias_base in (
                        (wk_sb, 2, 2), (wq_sb, 0, 0)
                    ):
                        for dt in range(NH):
                            psq = ps2.tile([128, QB], F32, name="psq", tag="psq")
                            for et in range(NET):
                                nc.tensor.matmul(
                                    psq[:, :],
                                    lhsT=w_sb[:, et * DL + dt * 128 : et * DL + (dt + 1) * 128],
                                    rhs=xt_sb[:, et * S + sb * QB : et * S + (sb + 1) * QB],
                                    start=(et == 0),
                                    stop=(et == NET - 1),
                                )
                            nc.scalar.activation(
                                qkt[:, (base4 + dt) * S + sb * QB : (base4 + dt) * S + (sb + 1) * QB],
                                psq[:, :],
                                mybir.ActivationFunctionType.Identity,
                                bias=biasqk[:, bias_base + dt : bias_base + dt + 1],
                                scale=1.0,
                            )

            # ------- Phases 3+4: attention with interleaved O-projection ---
            with tc.tile_pool(name="mid", bufs=1) as mid, \
                 tc.tile_pool(name="psO", bufs=2, space="PSUM") as psO_p, \
                 tc.tile_pool(name="psD", bufs=1, space="PSUM") as psD_p, \
                 tc.tile_pool(name="psS", bufs=2, space="PSUM") as psS_p, \
                 tc.tile_pool(name="psF", bufs=1, space="PSUM") as psF_p, \
                 tc.tile_pool(name="pp", bufs=6) as pp, \
                 tc.tile_pool(name="rp", bufs=2) as rp, \
                 tc.tile_pool(name="rbp", bufs=2) as rbp, \
                 tc.tile_pool(name="op", bufs=4) as op:
                # normalized attention outputs, transposed: (h*NQB+qb) tile [128d, 512q]
                outt = mid.tile([128, NH * NQB * QB], BF16, name="outt")
                wot_sb = mid.tile([128, NH * D], BF16, name="wot_sb")
                nc.sync.dma_start(
                    out=wot_sb.rearrange("p (h e) -> p h e", h=NH),
                    in_=WOT.rearrange("(h p) e -> p h e", p=128),
                )

                osb_open: dict = {}

                def emit_proj(qb0: int, j: int, et: int, alt: bool = False):
                    # O-projection for s-tile (qb0,j), e-chunk et; both heads
                    # accumulate in one psF bank, drained to bf16.  The four
                    # e-chunks of an s-tile share one osb staging tile so each
                    # s-tile costs a single output DMA.  In the tail (alt),
                    # items alternate into the psD bank (free after the last
                    # normalize) for a 2-deep psF rotation.
                    st = qb0 * 4 + j
                    if alt:
                        psF = psD_p.tile([128, 512], F32, name="psFt", tag="psD")
                    else:
                        psF = psF_p.tile([128, 512], F32, name="psF", tag="psF")
                    for h in range(NH):
                        o_base = (h * NQB + qb0) * QB + j * 128
                        nc.tensor.matmul(
                            psF[:, :],
                            lhsT=outt[:, o_base : o_base + 128],
                            rhs=wot_sb[:, h * D + et * 512 : h * D + (et + 1) * 512],
                            start=(h == 0),
                            stop=(h == NH - 1),
                        )
                    if st not in osb_open:
                        osb_open[st] = op.tile([128, D], BF16, name="osb", tag="osb")
                    osb = osb_open[st]
                    nc.vector.tensor_copy(
                        osb[:, et * 512 : (et + 1) * 512], psF[:, :]
                    )
                    if et == 3:
                        nc.sync.dma_start(
                            out=OUT[st * 128 : (st + 1) * 128, :],
                            in_=osb[:, :],
                        )
                        del osb_open[st]

                proj_items: list = []

                for qb in range(NQB):
                    n_k = 4 * (qb + 1) if is_causal else NKT
                    psO = {}
                    for h in range(NH):
                        psO[h] = psO_p.tile([128, QB], F32, name="psO", tag="psO")
                    psD = psD_p.tile([128, QB], F32, name="psD", tag="psD")
                    npairs = n_k // 2
                    # spread the previous q-block's O-projection through this
                    # q-block's pairs so ready matmuls fill exp-wait stalls
                    per_pair = -(-len(proj_items) // npairs) if proj_items else 0

                    for pi in range(npairs):
                        kt0 = 2 * pi
                        # scores + exp for both heads first (ACT gets a head
                        # start while the PE streams the other head's scores)
                        ps_info = {}
                        for h in range(NH):
                            diag = is_causal and (kt0 + 1 >= 4 * qb)
                            offs = (
                                (128 * (kt0 - 4 * qb), 128 * (kt0 + 1 - 4 * qb))
                                if diag
                                else (0, 0)
                            )
                            p = pp.tile([128, 2 * QB], BF16, name="p", tag="p")
                            psS = psS_p.tile([128, 2 * QB], F32, name="psS", tag="psS")
                            if not diag:
                                for u in range(2):
                                    kt = kt0 + u
                                    nc.tensor.matmul(
                                        psS[:, u * QB : (u + 1) * QB],
                                        lhsT=qkt[:, (2 + h) * S + kt * 128 : (2 + h) * S + (kt + 1) * 128],
                                        rhs=qkt[:, h * S + qb * QB : h * S + (qb + 1) * QB],
                                        start=True,
                                        stop=True,
                                    )
                                # one exp instruction for both k-tiles — ACT
                                # per-instruction overhead is ~40% of a
                                # 512-col activation
                                nc.scalar.activation(
                                    p[:, :], psS[:, :],
                                    mybir.ActivationFunctionType.Exp,
                                    scale=float(SCALE),
                                )
                            else:
                                for u in range(2):
                                    kt = kt0 + u
                                    off = offs[u]
                                    w = QB - off
                                    # scores for valid q-cols only, written at
                                    # bank-aligned offset 0 of this half
                                    nc.tensor.matmul(
                                        psS[:, u * QB : u * QB + w],
                                        lhsT=qkt[:, (2 + h) * S + kt * 128 : (2 + h) * S + (kt + 1) * 128],
                                        rhs=qkt[:, h * S + qb * QB + off : h * S + (qb + 1) * QB],
                                        start=True,
                                        stop=False,
                                    )
                                    # additive triangle on the first 128 valid
                                    # cols via identity-weight matmul
                                    nc.tensor.matmul(
                                        psS[:, u * QB : u * QB + 128],
                                        lhsT=masks_sb[:, 128:256],
                                        rhs=masks_sb[:, 0:128],
                                        start=False,
                                        stop=True,
                                    )
                                    # exp lands q-aligned in p; pad cols are
                                    # never read (attn@V / denom are narrowed)
                                    nc.scalar.activation(
                                        p[:, u * QB + off : (u + 1) * QB],
                                        psS[:, u * QB : u * QB + w],
                                        mybir.ActivationFunctionType.Exp,
                                        scale=float(SCALE),
                                    )
                            ps_info[h] = (p, offs)
                        # attn@V + denominators, narrowed to the causally
                        # valid q-columns on diagonal k-tiles
                        for h in range(NH):
                            p, offs = ps_info[h]
                            for u in range(2):
                                kt = kt0 + u
                                off = offs[u]
                                nc.tensor.matmul(
                                    psO[h][:, off:QB],
                                    lhsT=vt[:, kt * DL + h * 128 : kt * DL + (h + 1) * 128],
                                    rhs=p[:, u * QB + off : (u + 1) * QB],
                                    start=(kt == 0),
                                    stop=(kt == n_k - 1),
                                )
                            # denominator: DVE pre-sums the two k-tiles (bf16,
                            # off the critical path), then ONE ones-matmul per
                            # pair — halves the PE denominator column count
                            off0, off1 = offs
                            psum2 = pp.tile([128, QB], BF16, name="psum2", tag="ps2", bufs=3)
                            if off1 > off0:
                                # u0's exclusive strip, then the common range
                                nc.vector.tensor_copy(
                                    psum2[:, off0:off1], p[:, off0:off1]
                                )
                                nc.vector.tensor_add(
                                    psum2[:, off1:QB],
                                    p[:, off1:QB],
                                    p[:, QB + off1 : 2 * QB],
                                )
                            else:
                                nc.vector.tensor_add(
                                    psum2[:, :], p[:, 0:QB], p[:, QB : 2 * QB]
                                )
                            nc.tensor.matmul(
                                psD[32 * h : 32 * h + 1, off0:QB],
                                lhsT=ones_col[:, :],
                                rhs=psum2[:, off0:QB],
                                start=(kt0 == 0),
                                stop=(kt0 + 2 >= n_k),
                                tile_position=(0, 32 * h),
                            )
                        # deferred O-projection right before the next pair's
                        # scores — ready matmuls sit exactly where the psS
                        # WAR (exp completion) stall would otherwise land
                        for _ in range(per_pair):
                            if proj_items:
                                emit_proj(*proj_items.pop(0))

                    # normalize: 1/denom (fast approx), broadcast via K=1
                    # matmul into the psF bank, scale psO into outt (bf16)
                    for h in range(NH):
                        # normalize chain stays off the ACT engine — ACT is
                        # the attention-phase pacer (exp).  NOTE: the custom
                        # reciprocal_approx_fast DVE op reads garbage from
                        # PSUM on hardware (CoreSim accepts it) — stage the
                        # denominator row through SBUF first.
                        dsb = rp.tile([1, QB], F32, name="dsb", tag="dsb")
                        nc.vector.tensor_copy(dsb[:, :], psD[32 * h : 32 * h + 1, :])
                        recipf = rp.tile([1, QB], F32, name="recipf", tag="recipf")
                        nc.vector.reciprocal_approx_fast(
                            out=recipf[:, :], in_=dsb[:, :]
                        )
                        # broadcast 1/denom across partitions on the idle
                        # GpSimd engine (SBUF->SBUF, no PSUM involved) —
                        # keeps the whole chain off PE and mostly off DVE
                        rb = rbp.tile([128, QB], F32, name="rb", tag="rb")
                        nc.gpsimd.partition_broadcast(rb[:, :], recipf[:, :])
                        o_base = (h * NQB + qb) * QB
                        nc.vector.tensor_mul(
                            outt[:, o_base : o_base + QB], psO[h][:, :], rb[:, :]
                        )
                    # flush any leftovers, then queue this block's O-proj
                    while proj_items:
                        emit_proj(*proj_items.pop(0))
                    proj_items = [(qb, j, et) for j in range(4) for et in range(4)]

                # tail: O-projection of the last q-block, alternating PSUM
                # banks so drains overlap the next pair of matmuls
                ti = 0
                while proj_items:
                    emit_proj(*proj_items.pop(0), alt=(ti % 2 == 1))
                    ti += 1
    nc.finalize()
    return nc


def _bf16(a: np.ndarray) -> np.ndarray:
    return np.ascontiguousarray(a.astype(ml_dtypes.bfloat16))


def make_in_maps(X, Wq, bq, Wk, bk, Wv, bv, Wo, is_causal: bool):
    x2d = np.asarray(X, dtype=np.float32).reshape(S, D)
    xt = _bf16(x2d.T)
    masks = np.zeros((2, 128, 128), dtype=ml_dtypes.bfloat16)
    if is_causal:
        ki = np.arange(128)[:, None]
        cj = np.arange(128)[None, :]
        masks[0] = np.where(ki <= cj, 0.0, -1e9).astype(ml_dtypes.bfloat16)
        masks[1] = np.eye(128, dtype=ml_dtypes.bfloat16)

    in_maps = []
    for c in range(NCORES):
        sl = slice(c * DL, (c + 1) * DL)
        in_maps.append(
            {
                "xt": xt,
                "wqt": _bf16(np.asarray(Wq)[sl, :].T),
                "wkt": _bf16(np.asarray(Wk)[sl, :].T),
                "wvt": _bf16(np.asarray(Wv)[sl, :].T),
                "bqkc": np.ascontiguousarray(
                    np.stack(
                        [
                            np.asarray(bq, dtype=np.float32)[sl][:128],
                            np.asarray(bq, dtype=np.float32)[sl][128:],
                            np.asarray(bk, dtype=np.float32)[sl][:128],
                            np.asarray(bk, dtype=np.float32)[sl][128:],
                        ],
                        axis=1,
                    )
                ),
                "bvrow": _bf16(np.asarray(bv)[None, sl]),
                "wot": _bf16(np.asarray(Wo)[:, sl].T),
                "masks": masks,
            }
        )
    return in_maps


_NC_CACHE: dict = {}


def _get_nc(is_causal: bool) -> bass.Bass:
    if is_causal not in _NC_CACHE:
        _NC_CACHE[is_causal] = build_nc(is_causal)
    return _NC_CACHE[is_causal]


def kernel(X, Wq, bq, Wk, bk, Wv, bv, Wo, bo, is_causal, **run_kwargs):
    causal = bool(int(np.asarray(is_causal)))
    nc = _get_nc(causal)
    in_maps = make_in_maps(X, Wq, bq, Wk, bk, Wv, bv, Wo, causal)
    res = run_bass_kernel_spmd(nc, in_maps, core_ids=list(range(NCORES)), **run_kwargs)
    out = np.asarray(bo, dtype=np.float32)[None, :].repeat(S, axis=0)
    for c in range(NCORES):
        out += np.asarray(res.results[c]["out"], dtype=np.float32)
    return out.reshape(1, S, D)


# revision 36
# speedup vs baseline: 1.2032x; 1.0021x over previous
"""Trainium2 Bass kernel for nn_MultiHeadAttention (B=1, S=4096, D=2048, H=16, HD=128).

Sharding: tensor-parallel over heads — 2 heads per core on 8 NeuronCores.
Each core computes its 2 heads' Q/K/V projections, causal attention, and a
partial output projection (row-split Wo); the host sums the 8 partials and
adds the output bias (the all-reduce/unshard step).

Layout strategy (per core, all matmuls bf16 with fp32 PSUM accumulation):
  - X^T [2048, 4096] uploaded (e-major) so projections contract over e.
    DMA'd sb-major (512-seq-col blocks across all 16 e-tiles) so the first
    projection matmuls start after ~2 MB instead of after the full 16 MB.
  - Q, K produced transposed: QT/KT [d, s]. Scores computed transposed,
    S^T[k, q] = KT_tile^T @ QT, so p = exp(S^T) has k on partitions and
    attn@V needs no transpose.
  - Scores for 2 k-tiles land in one [128,1024] PSUM tile (2 banks) and get
    ONE exp instruction — halves the ACT per-instruction overhead.
  - Causal diagonal k-tiles stream only the valid q-columns (>= 128*jj into
    the q-block); the causal mask is a single shared [128,128] additive
    triangle applied via a 128-col identity matmul.
  - Softmax denominators accumulate on PE into one PSUM bank (h0 row 0,
    h1 row 32 via tile_position); 1/denom via reciprocal_approx_fast (DVE),
    broadcast across partitions with a K=1 matmul into the psF bank.
  - O-projection (row-split Wo, accumulated over both local heads) for
    q-block qb is interleaved into q-block qb+1's attention instruction
    stream, so exp-gated PE stalls are filled with ready matmuls.
  - Output partials are written bf16 (summed in fp32 on the host).

Built with bacc.Bacc (event-semaphore chains for multi-wait sync).
"""

import numpy as np
import ml_dtypes

import concourse.bass as bass
import concourse.mybir as mybir
import concourse.tile as tile
from concourse import bacc
from concourse.bass_utils import run_bass_kernel_spmd


S = 4096          # sequence length
D = 2048          # model dim
NCORES = 8
DL = D // NCORES  # 256 local head dims (2 heads)
NH = 2            # heads per core
HD = 128          # head dim
QB = 512          # q block width
NQB = S // QB     # 8
KT = 128          # k tile (partitions)
NKT = S // KT     # 32
ET = 128          # e contraction tile
NET = D // ET     # 16
NST = S // 128    # 32 s-tiles
SCALE = 1.0 / np.sqrt(HD)

BF16 = mybir.dt.bfloat16
F32 = mybir.dt.float32


def build_nc(is_causal: bool) -> bass.Bass:
    nc = bacc.Bacc()

    XT = nc.dram_tensor("xt", [D, S], BF16, kind="ExternalInput")
    WQT = nc.dram_tensor("wqt", [D, DL], BF16, kind="ExternalInput")
    WKT = nc.dram_tensor("wkt", [D, DL], BF16, kind="ExternalInput")
    WVT = nc.dram_tensor("wvt", [D, DL], BF16, kind="ExternalInput")
    # bias columns [128, 4]: bq.d0 | bq.d1 | bk.d0 | bk.d1
    BQKC = nc.dram_tensor("bqkc", [128, 4], F32, kind="ExternalInput")
    BVROW = nc.dram_tensor("bvrow", [1, DL], BF16, kind="ExternalInput")
    WOT = nc.dram_tensor("wot", [DL, D], BF16, kind="ExternalInput")
    # masks[0]: additive causal triangle (0 / -1e9); masks[1]: identity
    MASKS = nc.dram_tensor("masks", [2, 128, 128], BF16, kind="ExternalInput")
    OUT = nc.dram_tensor("out", [S, D], BF16, kind="ExternalOutput")

    with tile.TileContext(nc) as tc:
        with tc.tile_pool(name="persist", bufs=1) as persist:
            # Q head0 | Q head1 | K head0 | K head1, each [128, 4096]
            qkt = persist.tile([128, 4 * S], BF16, name="qkt")
            # V natural layout: s-tile st at cols [st*256, (st+1)*256), head h at +h*128
            vt = persist.tile([128, NST * DL], BF16, name="vt")
            ones_col = persist.tile([128, 1], BF16, name="ones_col")
            ones_row = persist.tile([1, 128], BF16, name="ones_row")
            biasqk = persist.tile([128, 4], F32, name="biasqk")
            bvrow_sb = persist.tile([1, DL], BF16, name="bvrow_sb")
            bvb_sb = persist.tile([128, DL], BF16, name="bvb_sb")
            masks_sb = persist.tile([128, 2 * 128], BF16, name="masks_sb")

            nc.vector.memset(ones_col[:, :], 1.0)
            nc.vector.memset(ones_row[:, :], 1.0)
            nc.sync.dma_start(out=bvrow_sb[:, :], in_=BVROW[:, :])
            nc.sync.dma_start(out=biasqk[:, :], in_=BQKC[:, :])
            if is_causal:
                # needed by q-block 0's diagonal matmuls — don't let it queue
                # behind the 16 MB X^T stream
                nc.sync.dma_start(
                    out=masks_sb.rearrange("p (j c) -> p j c", j=2),
                    in_=MASKS.rearrange("j p c -> p j c"),
                )

            # ---------------- Phase 2: QKV projections (sb-major) ----------
            with tc.tile_pool(name="xtp", bufs=1) as xtp, \
                 tc.tile_pool(name="wp", bufs=1) as wp, \
                 tc.tile_pool(name="ps2", bufs=3, space="PSUM") as ps2:
                xt_sb = xtp.tile([128, NET * S], BF16, name="xt_sb")
                wv_sb = wp.tile([128, NET * DL], BF16, name="wv_sb", tag="wv")
                wk_sb = wp.tile([128, NET * DL], BF16, name="wk_sb", tag="wk")
                wq_sb = wp.tile([128, NET * DL], BF16, name="wq_sb", tag="wq")
                # one strided DMA per transfer: DMA trigger instructions cost
                # ~0.6us each on the sync engine, so batching matters
                xt3 = xt_sb.rearrange("p (et s) -> p et s", et=NET)
                XT3 = XT.rearrange("(et p) s -> p et s", p=128)

                def dma_xt_block(sb):
                    nc.sync.dma_start(
                        out=xt3[:, :, sb * QB : (sb + 1) * QB],
                        in_=XT3[:, :, sb * QB : (sb + 1) * QB],
                    )

                nc.sync.dma_start(
                    out=wv_sb.rearrange("p (et d) -> p et d", et=NET),
                    in_=WVT.rearrange("(et p) d -> p et d", p=128),
                )
                # first s-block in 128-col chunks so the first V s-tile's
                # matmuls start as early as possible
                for st4 in range(4):
                    nc.sync.dma_start(
                        out=xt3[:, :, st4 * 128 : (st4 + 1) * 128],
                        in_=XT3[:, :, st4 * 128 : (st4 + 1) * 128],
                    )
                nc.sync.dma_start(
                    out=wk_sb.rearrange("p (et d) -> p et d", et=NET),
                    in_=WKT.rearrange("(et p) d -> p et d", p=128),
                )
                nc.sync.dma_start(
                    out=wq_sb.rearrange("p (et d) -> p et d", et=NET),
                    in_=WQT.rearrange("(et p) d -> p et d", p=128),
                )

                # broadcast bv across partitions once
                psb = ps2.tile([128, DL], F32, name="psb", tag="psv")
                nc.tensor.matmul(
                    psb[:, :], lhsT=ones_row[:, :], rhs=bvrow_sb[:, :],
                    start=True, stop=True,
                )
                nc.vector.tensor_copy(bvb_sb[:, :], psb[:, :])

                for sb in range(NQB):
                    if sb > 0:
                        dma_xt_block(sb)
                    # V for the 4 s-tiles of this block
                    for st4 in range(4):
                        st = 4 * sb + st4
                        psv = ps2.tile([128, DL], F32, name="psv", tag="psv")
                        for et in range(NET):
                            nc.tensor.matmul(
                                psv[:, :],
                                lhsT=xt_sb[:, et * S + st * 128 : et * S + (st + 1) * 128],
                                rhs=wv_sb[:, et * DL : (et + 1) * DL],
                                start=(et == 0),
                                stop=(et == NET - 1),
                            )
                        nc.vector.scalar_tensor_tensor(
                            out=vt[:, st * DL : (st + 1) * DL],
                            in0=psv[:, :],
                            scalar=1.0,
                            in1=bvb_sb[:, :],
                            op0=mybir.AluOpType.mult,
                            op1=mybir.AluOpType.add,
                        )
                    # K then Q for this block; bias fused into the ACT drain
                    for w_sb, base4, bias_base in (
                        (wk_sb, 2, 2), (wq_sb, 0, 0)
                    ):
                        for dt in range(NH):
                            psq = ps2.tile([128, QB], F32, name="psq", tag="psq")
                            for et in range(NET):
                                nc.tensor.matmul(
                                    psq[:, :],
                                    lhsT=w_sb[:, et * DL + dt * 128 : et * DL + (dt + 1) * 128],
                                    rhs=xt_sb[:, et * S + sb * QB : et * S + (sb + 1) * QB],
                                    start=(et == 0),
                                    stop=(et == NET - 1),
                                )
                            nc.scalar.activation(
                                qkt[:, (base4 + dt) * S + sb * QB : (base4 + dt) * S + (sb + 1) * QB],
                                psq[:, :],
                                mybir.ActivationFunctionType.Identity,
                                bias=biasqk[:, bias_base + dt : bias_base + dt + 1],
                                scale=1.0,
                            )

            # ------- Phases 3+4: attention with interleaved O-projection ---
            with tc.tile_pool(name="mid", bufs=1) as mid, \
                 tc.tile_pool(name="psO", bufs=2, space="PSUM") as psO_p, \
                 tc.tile_pool(name="psD", bufs=1, space="PSUM") as psD_p, \
                 tc.tile_pool(name="psS", bufs=2, space="PSUM") as psS_p, \
                 tc.tile_pool(name="psF", bufs=1, space="PSUM") as psF_p, \
                 tc.tile_pool(name="pp", bufs=10) as pp, \
                 tc.tile_pool(name="rp", bufs=2) as rp, \
                 tc.tile_pool(name="rbp", bufs=2) as rbp, \
                 tc.tile_pool(name="op", bufs=4) as op:
                # normalized attention outputs, transposed: (h*NQB+qb) tile [128d, 512q]
                outt = mid.tile([128, NH * NQB * QB], BF16, name="outt")
                wot_sb = mid.tile([128, NH * D], BF16, name="wot_sb")
                nc.sync.dma_start(
                    out=wot_sb.rearrange("p (h e) -> p h e", h=NH),
                    in_=WOT.rearrange("(h p) e -> p h e", p=128),
                )

                osb_open: dict = {}

                def emit_proj(qb0: int, j: int, et: int, alt: bool = False):
                    # O-projection for s-tile (qb0,j), e-chunk et; both heads
                    # accumulate in one psF bank, drained to bf16.  The four
                    # e-chunks of an s-tile share one osb staging tile so each
                    # s-tile costs a single output DMA.  In the tail (alt),
                    # items alternate into the psD bank (free after the last
                    # normalize) for a 2-deep psF rotation.
                    st = qb0 * 4 + j
                    if alt:
                        psF = psD_p.tile([128, 512], F32, name="psFt", tag="psD")
                    else:
                        psF = psF_p.tile([128, 512], F32, name="psF", tag="psF")
                    for h in range(NH):
                        o_base = (h * NQB + qb0) * QB + j * 128
                        nc.tensor.matmul(
                            psF[:, :],
                            lhsT=outt[:, o_base : o_base + 128],
                            rhs=wot_sb[:, h * D + et * 512 : h * D + (et + 1) * 512],
                            start=(h == 0),
                            stop=(h == NH - 1),
                        )
                    if st not in osb_open:
                        osb_open[st] = op.tile([128, D], BF16, name="osb", tag="osb")
                    osb = osb_open[st]
                    nc.vector.tensor_copy(
                        osb[:, et * 512 : (et + 1) * 512], psF[:, :]
                    )
                    if et == 3:
                        nc.sync.dma_start(
                            out=OUT[st * 128 : (st + 1) * 128, :],
                            in_=osb[:, :],
                        )
                        del osb_open[st]

                proj_items: list = []

                for qb in range(NQB):
                    n_k = 4 * (qb + 1) if is_causal else NKT
                    psO = {}
                    for h in range(NH):
                        psO[h] = psO_p.tile([128, QB], F32, name="psO", tag="psO")
                    psD = psD_p.tile([128, QB], F32, name="psD", tag="psD")
                    npairs = n_k // 2
                    # spread the previous q-block's O-projection through this
                    # q-block's pairs so ready matmuls fill exp-wait stalls
                    per_pair = -(-len(proj_items) // npairs) if proj_items else 0

                    for pi in range(npairs):
                        kt0 = 2 * pi
                        # scores + exp for both heads first (ACT gets a head
                        # start while the PE streams the other head's scores)
                        ps_info = {}
                        for h in range(NH):
                            diag = is_causal and (kt0 + 1 >= 4 * qb)
                            offs = (
                                (128 * (kt0 - 4 * qb), 128 * (kt0 + 1 - 4 * qb))
                                if diag
                                else (0, 0)
                            )
                            p = pp.tile([128, 2 * QB], BF16, name="p", tag="p")
                            psS = psS_p.tile([128, 2 * QB], F32, name="psS", tag="psS")
                            if not diag:
                                for u in range(2):
                                    kt = kt0 + u
                                    nc.tensor.matmul(
                                        psS[:, u * QB : (u + 1) * QB],
                                        lhsT=qkt[:, (2 + h) * S + kt * 128 : (2 + h) * S + (kt + 1) * 128],
                                        rhs=qkt[:, h * S + qb * QB : h * S + (qb + 1) * QB],
                                        start=True,
                                        stop=True,
                                    )
                                # one exp instruction for both k-tiles — ACT
                                # per-instruction overhead is ~40% of a
                                # 512-col activation
                                nc.scalar.activation(
                                    p[:, :], psS[:, :],
                                    mybir.ActivationFunctionType.Exp,
                                    scale=float(SCALE),
                                )
                            else:
                                for u in range(2):
                                    kt = kt0 + u
                                    off = offs[u]
                                    w = QB - off
                                    # scores for valid q-cols only, written at
                                    # bank-aligned offset 0 of this half
                                    nc.tensor.matmul(
                                        psS[:, u * QB : u * QB + w],
                                        lhsT=qkt[:, (2 + h) * S + kt * 128 : (2 + h) * S + (kt + 1) * 128],
                                        rhs=qkt[:, h * S + qb * QB + off : h * S + (qb + 1) * QB],
                                        start=True,
                                        stop=False,
                                    )
                                    # additive triangle on the first 128 valid
                                    # cols via identity-weight matmul
                                    nc.tensor.matmul(
                                        psS[:, u * QB : u * QB + 128],
                                        lhsT=masks_sb[:, 128:256],
                                        rhs=masks_sb[:, 0:128],
                                        start=False,
                                        stop=True,
                                    )
                                    # exp lands q-aligned in p; pad cols are
                                    # never read (attn@V / denom are narrowed)
                                    nc.scalar.activation(
                                        p[:, u * QB + off : (u + 1) * QB],
                                        psS[:, u * QB : u * QB + w],
                                        mybir.ActivationFunctionType.Exp,
                                        scale=float(SCALE),
                                    )
                            ps_info[h] = (p, offs)
                        # attn@V + denominators, narrowed to the causally
                        # valid q-columns on diagonal k-tiles
                        for h in range(NH):
                            p, offs = ps_info[h]
                            for u in range(2):
                                kt = kt0 + u
                                off = offs[u]
                                nc.tensor.matmul(
                                    psO[h][:, off:QB],
                                    lhsT=vt[:, kt * DL + h * 128 : kt * DL + (h + 1) * 128],
                                    rhs=p[:, u * QB + off : (u + 1) * QB],
                                    start=(kt == 0),
                                    stop=(kt == n_k - 1),
                                )
                            # denominator: DVE pre-sums the two k-tiles (bf16,
                            # off the critical path), then ONE ones-matmul per
                            # pair — halves the PE denominator column count
                            off0, off1 = offs
                            psum2 = pp.tile([128, QB], BF16, name="psum2", tag="ps2", bufs=4)
                            if off1 > off0:
                                # u0's exclusive strip, then the common range
                                nc.vector.tensor_copy(
                                    psum2[:, off0:off1], p[:, off0:off1]
                                )
                                nc.vector.tensor_add(
                                    psum2[:, off1:QB],
                                    p[:, off1:QB],
                                    p[:, QB + off1 : 2 * QB],
                                )
                            else:
                                nc.vector.tensor_add(
                                    psum2[:, :], p[:, 0:QB], p[:, QB : 2 * QB]
                                )
                            nc.tensor.matmul(
                                psD[32 * h : 32 * h + 1, off0:QB],
                                lhsT=ones_col[:, :],
                                rhs=psum2[:, off0:QB],
                                start=(kt0 == 0),
                                stop=(kt0 + 2 >= n_k),
                                tile_position=(0, 32 * h),
                            )
                        # deferred O-projection right before the next pair's
                        # scores — ready matmuls sit exactly where the psS
                        # WAR (exp completion) stall would otherwise land
                        for _ in range(per_pair):
                            if proj_items:
                                emit_proj(*proj_items.pop(0))

                    # normalize: 1/denom (fast approx), broadcast via K=1
                    # matmul into the psF bank, scale psO into outt (bf16)
                    for h in range(NH):
                        # normalize chain stays off the ACT engine — ACT is
                        # the attention-phase pacer (exp).  NOTE: the custom
                        # reciprocal_approx_fast DVE op reads garbage from
                        # PSUM on hardware (CoreSim accepts it) — stage the
                        # denominator row through SBUF first.
                        dsb = rp.tile([1, QB], F32, name="dsb", tag="dsb")
                        nc.vector.tensor_copy(dsb[:, :], psD[32 * h : 32 * h + 1, :])
                        recipf = rp.tile([1, QB], F32, name="recipf", tag="recipf")
                        nc.vector.reciprocal_approx_fast(
                            out=recipf[:, :], in_=dsb[:, :]
                        )
                        # broadcast 1/denom across partitions on the idle
                        # GpSimd engine (SBUF->SBUF, no PSUM involved) —
                        # keeps the whole chain off PE and mostly off DVE
                        rb = rbp.tile([128, QB], F32, name="rb", tag="rb")
                        nc.gpsimd.partition_broadcast(rb[:, :], recipf[:, :])
                        o_base = (h * NQB + qb) * QB
                        nc.vector.tensor_mul(
                            outt[:, o_base : o_base + QB], psO[h][:, :], rb[:, :]
                        )
                    # flush any leftovers, then queue this block's O-proj
                    while proj_items:
                        emit_proj(*proj_items.pop(0))
                    proj_items = [(qb, j, et) for j in range(4) for et in range(4)]

                # tail: O-projection of the last q-block, alternating PSUM
                # banks so drains overlap the next pair of matmuls
                ti = 0
                while proj_items:
                    emit_proj(*proj_items.pop(0), alt=(ti % 2 == 1))
                    ti += 1
    nc.finalize()
    return nc


def _bf16(a: np.ndarray) -> np.ndarray:
    return np.ascontiguousarray(a.astype(ml_dtypes.bfloat16))


def make_in_maps(X, Wq, bq, Wk, bk, Wv, bv, Wo, is_causal: bool):
    x2d = np.asarray(X, dtype=np.float32).reshape(S, D)
    xt = _bf16(x2d.T)
    masks = np.zeros((2, 128, 128), dtype=ml_dtypes.bfloat16)
    if is_causal:
        ki = np.arange(128)[:, None]
        cj = np.arange(128)[None, :]
        masks[0] = np.where(ki <= cj, 0.0, -1e9).astype(ml_dtypes.bfloat16)
        masks[1] = np.eye(128, dtype=ml_dtypes.bfloat16)

    in_maps = []
    for c in range(NCORES):
        sl = slice(c * DL, (c + 1) * DL)
        in_maps.append(
            {
                "xt": xt,
                "wqt": _bf16(np.asarray(Wq)[sl, :].T),
                "wkt": _bf16(np.asarray(Wk)[sl, :].T),
                "wvt": _bf16(np.asarray(Wv)[sl, :].T),
                "bqkc": np.ascontiguousarray(
                    np.stack(
                        [
                            np.asarray(bq, dtype=np.float32)[sl][:128],
                            np.asarray(bq, dtype=np.float32)[sl][128:],
                            np.asarray(bk, dtype=np.float32)[sl][:128],
                            np.asarray(bk, dtype=np.float32)[sl][128:],
                        ],
                        axis=1,
                    )
                ),
                "bvrow": _bf16(np.asarray(bv)[None, sl]),
                "wot": _bf16(np.asarray(Wo)[:, sl].T),
                "masks": masks,
            }
        )
    return in_maps


_NC_CACHE: dict = {}


def _get_nc(is_causal: bool) -> bass.Bass:
    if is_causal not in _NC_CACHE:
        _NC_CACHE[is_causal] = build_nc(is_causal)
    return _NC_CACHE[is_causal]


def kernel(X, Wq, bq, Wk, bk, Wv, bv, Wo, bo, is_causal, **run_kwargs):
    causal = bool(int(np.asarray(is_causal)))
    nc = _get_nc(causal)
    in_maps = make_in_maps(X, Wq, bq, Wk, bk, Wv, bv, Wo, causal)
    res = run_bass_kernel_spmd(nc, in_maps, core_ids=list(range(NCORES)), **run_kwargs)
    out = np.asarray(bo, dtype=np.float32)[None, :].repeat(S, axis=0)
    for c in range(NCORES):
        out += np.asarray(res.results[c]["out"], dtype=np.float32)
    return out.reshape(1, S, D)


# revision 38
# speedup vs baseline: 1.2604x; 1.0476x over previous
"""Trainium2 Bass kernel for nn_MultiHeadAttention (B=1, S=4096, D=2048, H=16, HD=128).

Sharding: tensor-parallel over heads — 2 heads per core on 8 NeuronCores.
Each core computes its 2 heads' Q/K/V projections, causal attention, and a
partial output projection (row-split Wo); the host sums the 8 partials and
adds the output bias (the all-reduce/unshard step).

Layout strategy (per core, all matmuls bf16 with fp32 PSUM accumulation):
  - X^T [2048, 4096] uploaded (e-major) so projections contract over e.
    DMA'd sb-major (512-seq-col blocks across all 16 e-tiles) so the first
    projection matmuls start after ~2 MB instead of after the full 16 MB.
  - Q, K produced transposed: QT/KT [d, s]. Scores computed transposed,
    S^T[k, q] = KT_tile^T @ QT, so p = exp(S^T) has k on partitions and
    attn@V needs no transpose.
  - Scores for 2 k-tiles land in one [128,1024] PSUM tile (2 banks) and get
    ONE exp instruction — halves the ACT per-instruction overhead.
  - Causal diagonal k-tiles stream only the valid q-columns (>= 128*jj into
    the q-block); the causal mask is a single shared [128,128] additive
    triangle applied via a 128-col identity matmul.
  - Softmax denominators accumulate on PE into one PSUM bank (h0 row 0,
    h1 row 32 via tile_position); 1/denom via reciprocal_approx_fast (DVE),
    broadcast across partitions with a K=1 matmul into the psF bank.
  - O-projection (row-split Wo, accumulated over both local heads) for
    q-block qb is interleaved into q-block qb+1's attention instruction
    stream, so exp-gated PE stalls are filled with ready matmuls.
  - Output partials are written bf16 (summed in fp32 on the host).

Built with bacc.Bacc (event-semaphore chains for multi-wait sync).
"""

import numpy as np
import ml_dtypes

import concourse.bass as bass
import concourse.mybir as mybir
import concourse.tile as tile
from concourse import bacc
from concourse.bass_utils import run_bass_kernel_spmd


S = 4096          # sequence length
D = 2048          # model dim
NCORES = 8
DL = D // NCORES  # 256 local head dims (2 heads)
NH = 2            # heads per core
HD = 128          # head dim
QB = 512          # q block width
NQB = S // QB     # 8
KT = 128          # k tile (partitions)
NKT = S // KT     # 32
ET = 128          # e contraction tile
NET = D // ET     # 16
NST = S // 128    # 32 s-tiles
SCALE = 1.0 / np.sqrt(HD)

BF16 = mybir.dt.bfloat16
F32 = mybir.dt.float32


def build_nc(is_causal: bool) -> bass.Bass:
    nc = bacc.Bacc()

    XT = nc.dram_tensor("xt", [D, S], BF16, kind="ExternalInput")
    WQT = nc.dram_tensor("wqt", [D, DL], BF16, kind="ExternalInput")
    WKT = nc.dram_tensor("wkt", [D, DL], BF16, kind="ExternalInput")
    WVT = nc.dram_tensor("wvt", [D, DL], BF16, kind="ExternalInput")
    # bias columns [128, 4]: bq.d0 | bq.d1 | bk.d0 | bk.d1
    BQKC = nc.dram_tensor("bqkc", [128, 4], F32, kind="ExternalInput")
    BVROW = nc.dram_tensor("bvrow", [1, DL], BF16, kind="ExternalInput")
    WOT = nc.dram_tensor("wot", [DL, D], BF16, kind="ExternalInput")
    # masks[0]: additive causal triangle (0 / -1e9); masks[1]: identity
    MASKS = nc.dram_tensor("masks", [2, 128, 128], BF16, kind="ExternalInput")
    OUT = nc.dram_tensor("out", [S, D], BF16, kind="ExternalOutput")

    with tile.TileContext(nc) as tc:
        with tc.tile_pool(name="persist", bufs=1) as persist:
            # Q head0 | Q head1 | K head0 | K head1, each [128, 4096]
            qkt = persist.tile([128, 4 * S], BF16, name="qkt")
            # V natural layout: s-tile st at cols [st*256, (st+1)*256), head h at +h*128
            vt = persist.tile([128, NST * DL], BF16, name="vt")
            ones_col = persist.tile([128, 1], BF16, name="ones_col")
            ones_row = persist.tile([1, 128], BF16, name="ones_row")
            biasqk = persist.tile([128, 4], F32, name="biasqk")
            bvrow_sb = persist.tile([1, DL], BF16, name="bvrow_sb")
            bvb_sb = persist.tile([128, DL], BF16, name="bvb_sb")
            masks_sb = persist.tile([128, 2 * 128], BF16, name="masks_sb")

            nc.vector.memset(ones_col[:, :], 1.0)
            nc.vector.memset(ones_row[:, :], 1.0)
            nc.sync.dma_start(out=bvrow_sb[:, :], in_=BVROW[:, :])
            nc.sync.dma_start(out=biasqk[:, :], in_=BQKC[:, :])
            if is_causal:
                # needed by q-block 0's diagonal matmuls — don't let it queue
                # behind the 16 MB X^T stream
                nc.sync.dma_start(
                    out=masks_sb.rearrange("p (j c) -> p j c", j=2),
                    in_=MASKS.rearrange("j p c -> p j c"),
                )

            # ---------------- Phase 2: QKV projections (sb-major) ----------
            with tc.tile_pool(name="xtp", bufs=1) as xtp, \
                 tc.tile_pool(name="wp", bufs=1) as wp, \
                 tc.tile_pool(name="ps2", bufs=3, space="PSUM") as ps2:
                xt_sb = xtp.tile([128, NET * S], BF16, name="xt_sb")
                wv_sb = wp.tile([128, NET * DL], BF16, name="wv_sb", tag="wv")
                wk_sb = wp.tile([128, NET * DL], BF16, name="wk_sb", tag="wk")
                wq_sb = wp.tile([128, NET * DL], BF16, name="wq_sb", tag="wq")
                # one strided DMA per transfer: DMA trigger instructions cost
                # ~0.6us each on the sync engine, so batching matters
                xt3 = xt_sb.rearrange("p (et s) -> p et s", et=NET)
                XT3 = XT.rearrange("(et p) s -> p et s", p=128)

                def dma_xt_block(sb):
                    nc.sync.dma_start(
                        out=xt3[:, :, sb * QB : (sb + 1) * QB],
                        in_=XT3[:, :, sb * QB : (sb + 1) * QB],
                    )

                nc.sync.dma_start(
                    out=wv_sb.rearrange("p (et d) -> p et d", et=NET),
                    in_=WVT.rearrange("(et p) d -> p et d", p=128),
                )
                # first s-block in 128-col chunks so the first V s-tile's
                # matmuls start as early as possible
                for st4 in range(4):
                    nc.sync.dma_start(
                        out=xt3[:, :, st4 * 128 : (st4 + 1) * 128],
                        in_=XT3[:, :, st4 * 128 : (st4 + 1) * 128],
                    )
                nc.sync.dma_start(
                    out=wk_sb.rearrange("p (et d) -> p et d", et=NET),
                    in_=WKT.rearrange("(et p) d -> p et d", p=128),
                )
                nc.sync.dma_start(
                    out=wq_sb.rearrange("p (et d) -> p et d", et=NET),
                    in_=WQT.rearrange("(et p) d -> p et d", p=128),
                )

                # broadcast bv across partitions once
                psb = ps2.tile([128, DL], F32, name="psb", tag="psv")
                nc.tensor.matmul(
                    psb[:, :], lhsT=ones_row[:, :], rhs=bvrow_sb[:, :],
                    start=True, stop=True,
                )
                nc.vector.tensor_copy(bvb_sb[:, :], psb[:, :])

                for sb in range(NQB):
                    if sb > 0:
                        dma_xt_block(sb)
                    # V for the 4 s-tiles of this block
                    for st4 in range(4):
                        st = 4 * sb + st4
                        psv = ps2.tile([128, DL], F32, name="psv", tag="psv")
                        for et in range(NET):
                            nc.tensor.matmul(
                                psv[:, :],
                                lhsT=xt_sb[:, et * S + st * 128 : et * S + (st + 1) * 128],
                                rhs=wv_sb[:, et * DL : (et + 1) * DL],
                                start=(et == 0),
                                stop=(et == NET - 1),
                            )
                        nc.vector.scalar_tensor_tensor(
                            out=vt[:, st * DL : (st + 1) * DL],
                            in0=psv[:, :],
                            scalar=1.0,
                            in1=bvb_sb[:, :],
                            op0=mybir.AluOpType.mult,
                            op1=mybir.AluOpType.add,
                        )
                    # K then Q for this block; bias fused into the ACT drain
                    for w_sb, base4, bias_base in (
                        (wk_sb, 2, 2), (wq_sb, 0, 0)
                    ):
                        for dt in range(NH):
                            psq = ps2.tile([128, QB], F32, name="psq", tag="psq")
                            for et in range(NET):
                                nc.tensor.matmul(
                                    psq[:, :],
                                    lhsT=w_sb[:, et * DL + dt * 128 : et * DL + (dt + 1) * 128],
                                    rhs=xt_sb[:, et * S + sb * QB : et * S + (sb + 1) * QB],
                                    start=(et == 0),
                                    stop=(et == NET - 1),
                                )
                            nc.scalar.activation(
                                qkt[:, (base4 + dt) * S + sb * QB : (base4 + dt) * S + (sb + 1) * QB],
                                psq[:, :],
                                mybir.ActivationFunctionType.Identity,
                                bias=biasqk[:, bias_base + dt : bias_base + dt + 1],
                                scale=1.0,
                            )

            # ------- Phases 3+4: attention with interleaved O-projection ---
            with tc.tile_pool(name="mid", bufs=1) as mid, \
                 tc.tile_pool(name="psO", bufs=2, space="PSUM") as psO_p, \
                 tc.tile_pool(name="psD", bufs=1, space="PSUM") as psD_p, \
                 tc.tile_pool(name="psS", bufs=4, space="PSUM") as psS_p, \
                 tc.tile_pool(name="psF", bufs=1, space="PSUM") as psF_p, \
                 tc.tile_pool(name="pp", bufs=10) as pp, \
                 tc.tile_pool(name="rp", bufs=2) as rp, \
                 tc.tile_pool(name="rbp", bufs=2) as rbp, \
                 tc.tile_pool(name="op", bufs=4) as op:
                # normalized attention outputs, transposed: (h*NQB+qb) tile [128d, 512q]
                outt = mid.tile([128, NH * NQB * QB], BF16, name="outt")
                wot_sb = mid.tile([128, NH * D], BF16, name="wot_sb")
                nc.sync.dma_start(
                    out=wot_sb.rearrange("p (h e) -> p h e", h=NH),
                    in_=WOT.rearrange("(h p) e -> p h e", p=128),
                )

                osb_open: dict = {}

                def emit_proj(qb0: int, j: int, et: int, alt: bool = False):
                    # O-projection for s-tile (qb0,j), e-chunk et; both heads
                    # accumulate in one psF bank, drained to bf16.  The four
                    # e-chunks of an s-tile share one osb staging tile so each
                    # s-tile costs a single output DMA.  In the tail (alt),
                    # items alternate into the psD bank (free after the last
                    # normalize) for a 2-deep psF rotation.
                    st = qb0 * 4 + j
                    if alt:
                        psF = psD_p.tile([128, 512], F32, name="psFt", tag="psD")
                    else:
                        psF = psF_p.tile([128, 512], F32, name="psF", tag="psF")
                    for h in range(NH):
                        o_base = (h * NQB + qb0) * QB + j * 128
                        nc.tensor.matmul(
                            psF[:, :],
                            lhsT=outt[:, o_base : o_base + 128],
                            rhs=wot_sb[:, h * D + et * 512 : h * D + (et + 1) * 512],
                            start=(h == 0),
                            stop=(h == NH - 1),
                        )
                    if st not in osb_open:
                        osb_open[st] = op.tile([128, D], BF16, name="osb", tag="osb")
                    osb = osb_open[st]
                    nc.vector.tensor_copy(
                        osb[:, et * 512 : (et + 1) * 512], psF[:, :]
                    )
                    if et == 3:
                        nc.sync.dma_start(
                            out=OUT[st * 128 : (st + 1) * 128, :],
                            in_=osb[:, :],
                        )
                        del osb_open[st]

                proj_items: list = []

                for qb in range(NQB):
                    n_k = 4 * (qb + 1) if is_causal else NKT
                    psO = {}
                    for h in range(NH):
                        psO[h] = psO_p.tile([128, QB], F32, name="psO", tag="psO")
                    psD = psD_p.tile([128, QB], F32, name="psD", tag="psD")
                    npairs = n_k // 2
                    # spread the previous q-block's O-projection through this
                    # q-block's pairs so ready matmuls fill exp-wait stalls
                    per_pair = -(-len(proj_items) // npairs) if proj_items else 0

                    for pi in range(npairs):
                        kt0 = 2 * pi
                        # scores + exp for both heads first (ACT gets a head
                        # start while the PE streams the other head's scores)
                        ps_info = {}
                        for h in range(NH):
                            diag = is_causal and (kt0 + 1 >= 4 * qb)
                            offs = (
                                (128 * (kt0 - 4 * qb), 128 * (kt0 + 1 - 4 * qb))
                                if diag
                                else (0, 0)
                            )
                            p = pp.tile([128, 2 * QB], BF16, name="p", tag="p")
                            # one psS bank + one exp per k-tile: a 4-deep psS
                            # ring decouples the PE from exp completion better
                            # than 2 two-bank megatiles
                            for u in range(2):
                                kt = kt0 + u
                                off = offs[u]
                                w = QB - off
                                psS = psS_p.tile([128, QB], F32, name="psS", tag="psS")
                                nc.tensor.matmul(
                                    psS[:, 0:w],
                                    lhsT=qkt[:, (2 + h) * S + kt * 128 : (2 + h) * S + (kt + 1) * 128],
                                    rhs=qkt[:, h * S + qb * QB + off : h * S + (qb + 1) * QB],
                                    start=True,
                                    stop=not diag,
                                )
                                if diag:
                                    # additive triangle on the first 128 valid
                                    # cols via identity-weight matmul
                                    nc.tensor.matmul(
                                        psS[:, 0:128],
                                        lhsT=masks_sb[:, 128:256],
                                        rhs=masks_sb[:, 0:128],
                                        start=False,
                                        stop=True,
                                    )
                                # exp lands q-aligned in p; pad cols are
                                # never read (attn@V / denom are narrowed)
                                nc.scalar.activation(
                                    p[:, u * QB + off : (u + 1) * QB],
                                    psS[:, 0:w],
                                    mybir.ActivationFunctionType.Exp,
                                    scale=float(SCALE),
                                )
                            ps_info[h] = (p, offs)
                        # attn@V + denominators, narrowed to the causally
                        # valid q-columns on diagonal k-tiles
                        for h in range(NH):
                            p, offs = ps_info[h]
                            for u in range(2):
                                kt = kt0 + u
                                off = offs[u]
                                nc.tensor.matmul(
                                    psO[h][:, off:QB],
                                    lhsT=vt[:, kt * DL + h * 128 : kt * DL + (h + 1) * 128],
                                    rhs=p[:, u * QB + off : (u + 1) * QB],
                                    start=(kt == 0),
                                    stop=(kt == n_k - 1),
                                )
                            # denominator: DVE pre-sums the two k-tiles (bf16,
                            # off the critical path), then ONE ones-matmul per
                            # pair — halves the PE denominator column count
                            off0, off1 = offs
                            psum2 = pp.tile([128, QB], BF16, name="psum2", tag="ps2", bufs=4)
                            if off1 > off0:
                                # u0's exclusive strip, then the common range
                                nc.vector.tensor_copy(
                                    psum2[:, off0:off1], p[:, off0:off1]
                                )
                                nc.vector.tensor_add(
                                    psum2[:, off1:QB],
                                    p[:, off1:QB],
                                    p[:, QB + off1 : 2 * QB],
                                )
                            else:
                                nc.vector.tensor_add(
                                    psum2[:, :], p[:, 0:QB], p[:, QB : 2 * QB]
                                )
                            nc.tensor.matmul(
                                psD[32 * h : 32 * h + 1, off0:QB],
                                lhsT=ones_col[:, :],
                                rhs=psum2[:, off0:QB],
                                start=(kt0 == 0),
                                stop=(kt0 + 2 >= n_k),
                                tile_position=(0, 32 * h),
                            )
                        # deferred O-projection right before the next pair's
                        # scores — ready matmuls sit exactly where the psS
                        # WAR (exp completion) stall would otherwise land
                        for _ in range(per_pair):
                            if proj_items:
                                emit_proj(*proj_items.pop(0))

                    # normalize: 1/denom (fast approx), broadcast via K=1
                    # matmul into the psF bank, scale psO into outt (bf16)
                    for h in range(NH):
                        # normalize chain stays off the ACT engine — ACT is
                        # the attention-phase pacer (exp).  NOTE: the custom
                        # reciprocal_approx_fast DVE op reads garbage from
                        # PSUM on hardware (CoreSim accepts it) — stage the
                        # denominator row through SBUF first.
                        dsb = rp.tile([1, QB], F32, name="dsb", tag="dsb")
                        nc.vector.tensor_copy(dsb[:, :], psD[32 * h : 32 * h + 1, :])
                        recipf = rp.tile([1, QB], F32, name="recipf", tag="recipf")
                        nc.vector.reciprocal_approx_fast(
                            out=recipf[:, :], in_=dsb[:, :]
                        )
                        # broadcast 1/denom across partitions on the idle
                        # GpSimd engine (SBUF->SBUF, no PSUM involved) —
                        # keeps the whole chain off PE and mostly off DVE
                        rb = rbp.tile([128, QB], F32, name="rb", tag="rb")
                        nc.gpsimd.partition_broadcast(rb[:, :], recipf[:, :])
                        o_base = (h * NQB + qb) * QB
                        nc.vector.tensor_mul(
                            outt[:, o_base : o_base + QB], psO[h][:, :], rb[:, :]
                        )
                    # flush any leftovers, then queue this block's O-proj
                    while proj_items:
                        emit_proj(*proj_items.pop(0))
                    proj_items = [(qb, j, et) for j in range(4) for et in range(4)]

                # tail: O-projection of the last q-block, alternating PSUM
                # banks so drains overlap the next pair of matmuls
                ti = 0
                while proj_items:
                    emit_proj(*proj_items.pop(0), alt=(ti % 2 == 1))
                    ti += 1
    nc.finalize()
    return nc


def _bf16(a: np.ndarray) -> np.ndarray:
    return np.ascontiguousarray(a.astype(ml_dtypes.bfloat16))


def make_in_maps(X, Wq, bq, Wk, bk, Wv, bv, Wo, is_causal: bool):
    x2d = np.asarray(X, dtype=np.float32).reshape(S, D)
    xt = _bf16(x2d.T)
    masks = np.zeros((2, 128, 128), dtype=ml_dtypes.bfloat16)
    if is_causal:
        ki = np.arange(128)[:, None]
        cj = np.arange(128)[None, :]
        masks[0] = np.where(ki <= cj, 0.0, -1e9).astype(ml_dtypes.bfloat16)
        masks[1] = np.eye(128, dtype=ml_dtypes.bfloat16)

    in_maps = []
    for c in range(NCORES):
        sl = slice(c * DL, (c + 1) * DL)
        in_maps.append(
            {
                "xt": xt,
                "wqt": _bf16(np.asarray(Wq)[sl, :].T),
                "wkt": _bf16(np.asarray(Wk)[sl, :].T),
                "wvt": _bf16(np.asarray(Wv)[sl, :].T),
                "bqkc": np.ascontiguousarray(
                    np.stack(
                        [
                            np.asarray(bq, dtype=np.float32)[sl][:128],
                            np.asarray(bq, dtype=np.float32)[sl][128:],
                            np.asarray(bk, dtype=np.float32)[sl][:128],
                            np.asarray(bk, dtype=np.float32)[sl][128:],
                        ],
                        axis=1,
                    )
                ),
                "bvrow": _bf16(np.asarray(bv)[None, sl]),
                "wot": _bf16(np.asarray(Wo)[:, sl].T),
                "masks": masks,
            }
        )
    return in_maps


_NC_CACHE: dict = {}


def _get_nc(is_causal: bool) -> bass.Bass:
    if is_causal not in _NC_CACHE:
        _NC_CACHE[is_causal] = build_nc(is_causal)
    return _NC_CACHE[is_causal]


def kernel(X, Wq, bq, Wk, bk, Wv, bv, Wo, bo, is_causal, **run_kwargs):
    causal = bool(int(np.asarray(is_causal)))
    nc = _get_nc(causal)
    in_maps = make_in_maps(X, Wq, bq, Wk, bk, Wv, bv, Wo, causal)
    res = run_bass_kernel_spmd(nc, in_maps, core_ids=list(range(NCORES)), **run_kwargs)
    out = np.asarray(bo, dtype=np.float32)[None, :].repeat(S, axis=0)
    for c in range(NCORES):
        out += np.asarray(res.results[c]["out"], dtype=np.float32)
    return out.reshape(1, S, D)
